# revision 1
# baseline (speedup 1.0000x reference)
"""Multi-head causal attention (B=2, S=2048, D=1024, H=16, DH=64) on 8 NeuronCores.

Sharding: data-parallel over batch (2) x tensor-parallel over heads (4 groups
of 4 heads). Core c handles batch c//4, heads 4*(c%4)..4*(c%4)+3. Each core
computes its head-group's Q/K/V projections, causal softmax attention, and a
partial output projection (Wo row-shard); the host sums the 4 partials per
batch.

Device-side layout choices:
- Inputs are uploaded transposed (xT: [D, S]) and in bf16 so every projection
  matmul contracts over the partition dim with contiguous DMA.
- q/k are produced transposed ([dh, s]); scores are computed transposed
  ([k, q]) so the softmax sum folds into the attention*V matmul via an extra
  ones-column on V, and the context comes out as ctxT [dh, q], which is
  exactly the stationary operand the Wo matmul needs.
- Normalization (divide by softmax sum) happens on ctxT via reciprocal +
  a rank-1 PE broadcast matmul; it is emitted one head late so the PE always
  has the next head's score matmuls available while the reciprocal runs.
- Score tiles are paired two k-chunks per PSUM tile so one Exp activation
  (and one diagonal-mask multiply) covers both.
"""

import numpy as np
import ml_dtypes

import concourse.bass as bass  # noqa: F401
import concourse.mybir as mybir
import concourse.tile as tile
from concourse import bacc
from concourse.bass_utils import run_bass_kernel_spmd

B, S, D, H, DH = 2, 2048, 1024, 16, 64
N_CORES = 8
HPC = 4            # heads per core
DG = HPC * DH      # 256 head dims per core
QW = 512           # q-chunk width
NQ = S // QW       # 4 q-chunks
NKC = S // 128     # 16 k-chunks
NDC = D // 128     # 8 contraction chunks for projections

BF = mybir.dt.bfloat16
F32 = mybir.dt.float32
F32R = mybir.dt.float32r

_CACHE = {}


def _emit(nc):
    xqT = nc.dram_tensor("xqT", [D, S], BF, kind="ExternalInput")
    xkT = nc.dram_tensor("xkT", [D, S], BF, kind="ExternalInput")
    xvT = nc.dram_tensor("xvT", [D, S], BF, kind="ExternalInput")
    wqT = nc.dram_tensor("wqT", [D, DG], BF, kind="ExternalInput")
    wkT = nc.dram_tensor("wkT", [D, DG], BF, kind="ExternalInput")
    wvT = nc.dram_tensor("wvT", [D, DG], BF, kind="ExternalInput")
    woT = nc.dram_tensor("woT", [DG, D], BF, kind="ExternalInput")
    mskd = nc.dram_tensor("msk", [128, 4, QW], BF, kind="ExternalInput")
    onesd = nc.dram_tensor("ones", [1, 64], F32R, kind="ExternalInput")
    outp = nc.dram_tensor("outp", [S, D], BF, kind="ExternalOutput")

    EXP = mybir.ActivationFunctionType.Exp

    with tile.TileContext(nc) as tc:
        with (
            tc.tile_pool(name="wpool", bufs=1) as wpool,
            tc.tile_pool(name="spool", bufs=1) as spool,
            tc.tile_pool(name="xpool", bufs=2) as xpool,
            tc.tile_pool(name="apool", bufs=10) as apool,
            tc.tile_pool(name="cpool", bufs=6) as cpool,
            tc.tile_pool(name="rpool", bufs=4) as rpool,
            tc.tile_pool(name="opool", bufs=6) as opool,
            tc.tile_pool(name="ppair", bufs=2, space="PSUM") as ppair,
            tc.tile_pool(name="pmain", bufs=2, space="PSUM") as pmain,
            tc.tile_pool(name="pctx", bufs=2, space="PSUM") as pctx,
        ):
            # --- persistent tiles ---
            wq = wpool.tile([128, NDC, DG], BF)
            wk = wpool.tile([128, NDC, DG], BF)
            wv = wpool.tile([128, NDC, DG], BF)
            wo = wpool.tile([128, 2, D], BF)
            msk = wpool.tile([128, 4, QW], BF)
            ones = wpool.tile([1, 64], F32R)
            qT = spool.tile([128, 2, S], BF)
            kT = spool.tile([128, 2, S], BF)
            vv = spool.tile([128, NKC, HPC, DH + 1], BF)
            ctxT = spool.tile([128, 2, S], BF)

            nc.sync.dma_start(wq[:], wqT.ap().rearrange("(c p) n -> p c n", p=128))
            nc.scalar.dma_start(wk[:], wkT.ap().rearrange("(c p) n -> p c n", p=128))
            nc.vector.memset(vv[:, :, :, DH : DH + 1], 1.0)

            # --- input DMAs (chunked, alternating the two HWDGE queues) ---
            dma_engines = (nc.sync, nc.scalar)
            xq = xpool.tile([128, NDC, S], BF, tag="xt")
            xk = xpool.tile([128, NDC, S], BF, tag="xt")
            for c in range(NDC):
                dma_engines[c % 2].dma_start(
                    xq[:, c, :], xqT.ap()[c * 128 : (c + 1) * 128, :]
                )
            for c in range(NDC):
                dma_engines[c % 2].dma_start(
                    xk[:, c, :], xkT.ap()[c * 128 : (c + 1) * 128, :]
                )

            for w, xt, dst in ((wq, xq, qT), (wk, xk, kT)):
                for t in range(2):
                    for si in range(NQ):
                        ps = pmain.tile([128, QW], F32, tag="ps")
                        for c in range(NDC):
                            nc.tensor.matmul(
                                ps[:],
                                w[:, c, t * 128 : (t + 1) * 128],
                                xt[:, c, si * QW : (si + 1) * QW],
                                start=(c == 0),
                                stop=(c == NDC - 1),
                            )
                        with tc.high_priority(offset=48):
                            nc.scalar.copy(
                                dst[:, t, si * QW : (si + 1) * QW], ps[:]
                            )

            # late weight loads (not needed until v-proj / attention)
            nc.sync.dma_start(wv[:], wvT.ap().rearrange("(c p) n -> p c n", p=128))
            nc.scalar.dma_start(wo[:], woT.ap().rearrange("(c p) n -> p c n", p=128))
            nc.sync.dma_start(msk[:], mskd.ap())
            nc.sync.dma_start(ones[:], onesd.ap())

            # --- projection: v (natural [s, dh]) ---
            xv = xpool.tile([128, NDC, S], BF, tag="xt")
            for c in range(NDC):
                dma_engines[c % 2].dma_start(
                    xv[:, c, :], xvT.ap()[c * 128 : (c + 1) * 128, :]
                )

            def emit_vproj(st):
                ps = pmain.tile([128, DG], F32, tag="ps")
                for c in range(NDC):
                    nc.tensor.matmul(
                        ps[:],
                        xv[:, c, st * 128 : (st + 1) * 128],
                        wv[:, c, :],
                        start=(c == 0),
                        stop=(c == NDC - 1),
                    )
                nc.vector.tensor_copy(
                    vv[:, st, :, 0:DH],
                    ps[:].rearrange("p (h e) -> p h e", e=DH),
                )

            # --- attention + output projection, per q-chunk ---
            def emit_norm_a(qi, h, cps):
                # stage A: reciprocal of the sums row + lift ctx out of PSUM
                rc = rpool.tile([1, QW], F32R)
                with nc.allow_low_precision(reason="f32r bits ~ f32"):
                    nc.vector.reciprocal(rc[:], cps[DH : DH + 1, :])
                cu = cpool.tile([64, QW], BF)
                nc.vector.tensor_copy(cu[:], cps[0:DH, :])
                return rc, cu

            def emit_norm_b(qi, h, rc, cu):
                # stage B: rank-1 broadcast of 1/sum and the normalize multiply
                t, p0 = h // 2, 64 * (h % 2)
                q_sl = slice(qi * QW, (qi + 1) * QW)
                bps = pmain.tile([64, QW], F32, tag="ps")
                nc.tensor.matmul(bps[:], ones[:], rc[:], start=True, stop=True)
                nc.vector.tensor_mul(ctxT[p0 : p0 + 64, t, q_sl], cu[:], bps[:])

            def emit_wo(qt):
                # 1-bank accumulators from pmain so Wo never occupies the
                # score-pair slots that feed the exp stream
                ob = opool.tile([128, D], BF)
                for nh in range(2):
                    ops = pmain.tile([128, 512], F32, tag="ps")
                    for t in range(2):
                        nc.tensor.matmul(
                            ops[:],
                            ctxT[:, t, qt * 128 : (qt + 1) * 128],
                            wo[:, t, nh * 512 : (nh + 1) * 512],
                            start=(t == 0),
                            stop=(t == 1),
                        )
                    nc.vector.tensor_copy(ob[:, nh * 512 : (nh + 1) * 512], ops[:])
                dma_engines[qt % 2].dma_start(
                    outp.ap()[qt * 128 : (qt + 1) * 128, :], ob[:]
                )

            for qi in range(NQ):
                # v-projection chunks needed by this q-chunk's attention
                for st in range(4 * qi, 4 * qi + 4):
                    emit_vproj(st)
                q_sl = slice(qi * QW, (qi + 1) * QW)
                nk = (qi + 1) * 4
                pending = None
                for h in range(HPC):
                    if qi > 0 and h >= 1:
                        emit_wo((qi - 1) * 4 + h - 1)
                    t, p0 = h // 2, 64 * (h % 2)
                    cps = pctx.tile([DH + 1, QW], F32, tag="cps")
                    for pc in range(nk // 2):
                        d0 = 2 * pc - qi * 4
                        # far diagonal pair (d0=2): columns [0:256) are fully
                        # causal-masked for both halves; compute half-width.
                        c0 = 256 if d0 == 2 else 0
                        csl = slice(c0, QW)
                        sps = ppair.tile([128, 2, QW], F32, tag="sps")
                        with tc.high_priority(offset=96 if pc == 0 else 72):
                            for half in range(2):
                                kc = 2 * pc + half
                                nc.tensor.matmul(
                                    sps[:, half, csl],
                                    kT[p0 : p0 + 64, t, kc * 128 : (kc + 1) * 128],
                                    qT[p0 : p0 + 64, t, qi * QW + c0 : (qi + 1) * QW],
                                    start=True,
                                    stop=True,
                                )
                        at = apool.tile([128, 2, QW], BF, tag="at")
                        with tc.high_priority(offset=96 if pc == 0 else 72):
                            nc.scalar.activation(at[:, :, csl], sps[:, :, csl], EXP)
                        if d0 >= 0:
                            nc.vector.tensor_mul(
                                at[:, :, csl], at[:, :, csl], msk[:, d0 : d0 + 2, csl]
                            )
                        for half in range(2):
                            kc = 2 * pc + half
                            nc.tensor.matmul(
                                cps[:, csl],
                                vv[:, kc, h, :],
                                at[:, half, csl],
                                start=(kc == 0),
                                stop=(kc == nk - 1),
                            )
                    if pending is not None:
                        ph, prc, pcu = pending
                        emit_norm_b(qi, ph, prc, pcu)
                    rc, cu = emit_norm_a(qi, h, cps)
                    pending = (h, rc, cu)
                ph, prc, pcu = pending
                emit_norm_b(qi, ph, prc, pcu)
                if qi > 0:
                    emit_wo(qi * 4 - 1)
            for j in range(4):
                emit_wo(12 + j)


def build_program():
    if "nc" in _CACHE:
        return _CACHE["nc"]
    nc = bacc.Bacc(
        "TRN2", target_bir_lowering=False, debug=False, num_devices=N_CORES
    )
    _emit(nc)
    nc.compile()
    _CACHE["nc"] = nc
    return nc


def _prep_in_maps(query, key, value, Wq, Wk, Wv, Wo):
    bf = ml_dtypes.bfloat16
    scale = 1.0 / np.sqrt(np.float32(DH))

    p, i, j = np.ogrid[0:128, 0:4, 0:QW]
    msk = (j >= 128 * i + p).astype(bf)

    xT = {}
    for b in range(B):
        xT[("q", b)] = np.ascontiguousarray(query[b].T).astype(bf)
        xT[("k", b)] = np.ascontiguousarray(key[b].T).astype(bf)
        xT[("v", b)] = np.ascontiguousarray(value[b].T).astype(bf)

    in_maps = []
    for c in range(N_CORES):
        b, g = c // HPC, c % HPC
        rows = slice(g * DG, (g + 1) * DG)
        in_maps.append(
            {
                "xqT": xT[("q", b)],
                "xkT": xT[("k", b)],
                "xvT": xT[("v", b)],
                "wqT": np.ascontiguousarray((Wq[rows] * scale).T).astype(bf),
                "wkT": np.ascontiguousarray(Wk[rows].T).astype(bf),
                "wvT": np.ascontiguousarray(Wv[rows].T).astype(bf),
                "woT": np.ascontiguousarray(Wo[:, rows].T).astype(bf),
                "msk": msk,
                "ones": np.ones((1, 64), dtype=np.float32),
            }
        )
    return in_maps


def kernel(query, key, value, Wq, Wk, Wv, Wo):
    query = np.asarray(query, dtype=np.float32)
    key = np.asarray(key, dtype=np.float32)
    value = np.asarray(value, dtype=np.float32)
    Wq = np.asarray(Wq, dtype=np.float32)
    Wk = np.asarray(Wk, dtype=np.float32)
    Wv = np.asarray(Wv, dtype=np.float32)
    Wo = np.asarray(Wo, dtype=np.float32)

    nc = build_program()
    in_maps = _prep_in_maps(query, key, value, Wq, Wk, Wv, Wo)
    res = run_bass_kernel_spmd(
        nc, in_maps, core_ids=list(range(N_CORES)), trace=False
    )
    out = np.zeros((B, S, D), dtype=np.float32)
    for b in range(B):
        for g in range(HPC):
            out[b] += res.results[b * HPC + g]["outp"].astype(np.float32)
    return out



# revision 6
# speedup vs baseline: 1.3541x; 1.3541x over previous
"""Multi-head causal attention (B=2, S=2048, D=1024, H=16, DH=64) on 8 NeuronCores.

Sharding: data-parallel over batch (2) x tensor-parallel over heads (4 groups
of 4 heads). Core c handles batch c//4, heads 4*(c%4)..4*(c%4)+3. Host sums
the 4 Wo row-shard partials per batch.

Precision/engine plan (validated numerically against the reference):
- Q/K projections: direct fp8(e4m3) DoubleRow matmuls (K=256/instr). Host
  uploads x8 = fp8(x) and Wq*16 / Wk*32 in fp8; PSUM holds 16*q / 32*k and the
  DVE lift scales by 1/64 into fp8 score operands q/4 and k/2, so the q*k
  matmul directly yields q*k/8 (the softmax scale).
- Scores: fp8 DoubleRow, contraction dh=64 on a 64-partition slice with the
  second DR k-tile zeroed.
- V projection: 3-term compensated fp8 DoubleRow (x8@W64 + x8@dW + 16dx@4W)
  keeping v at ~bf16 accuracy; PSUM holds 64*v which cancels against the
  1/(64*den) reciprocal in the softmax normalization.
- attention*V, Wo: bf16 (fp8 here fails the error budget).
- exp on ACT; causal mask multiply on GPSIMD; 1/den broadcast by rank-1 PE
  matmul; PSUM->SBUF lifts and output copies on DVE.

Dataflow choices for pipeline fill: q/k (and v/dv) inputs ship as single
combined dram tensors DMA'd in column phases (cols 0:512 first) so the first
score pairs are ready ~5us in; projections emit si-major (q then k per si) so
attention on q-chunk 0 unblocks after 4 of 16 projection tiles; v-projection
chunks are spread inside the first head's pair loop of each q-chunk so the
ACT exp stream never starves behind a projection burst.
"""

import numpy as np
import ml_dtypes

import concourse.bass as bass  # noqa: F401
import concourse.mybir as mybir
import concourse.tile as tile
from concourse import bacc
from concourse.bass_utils import run_bass_kernel_spmd

B, S, D, H, DH = 2, 2048, 1024, 16, 64
N_CORES = 8
HPC = 4            # heads per core
DG = HPC * DH      # 256 head dims per core
QW = 512           # q-chunk width
NQ = S // QW       # 4 q-chunks
NCC = D // 256     # 4 DR contraction chunks for projections

BF = mybir.dt.bfloat16
F8 = mybir.dt.float8e4
F32 = mybir.dt.float32
F32R = mybir.dt.float32r
DR = mybir.MatmulPerfMode.DoubleRow

_CACHE = {}


def _emit(nc):
    # x layouts put the 512-wide column phase OUTERMOST so each phase DMA is
    # one contiguous per-partition subregion (cheap exact dependency ranges).
    xqkd = nc.dram_tensor("xqk8", [128, NQ, 2, NCC, 2, QW], F8, kind="ExternalInput")
    xvvd = nc.dram_tensor("xvv8", [128, NQ, 2, NCC, 2, QW], F8, kind="ExternalInput")
    wqkd = nc.dram_tensor("wqk8", [128, 2, 2, NCC, 2, 128], F8, kind="ExternalInput")
    wv3d = nc.dram_tensor("wv3", [128, 3, NCC, 2, DG], F8, kind="ExternalInput")
    wod = nc.dram_tensor("woT", [128, 2, D], BF, kind="ExternalInput")
    mskd = nc.dram_tensor("msk", [128, 4, QW], BF, kind="ExternalInput")
    outp = nc.dram_tensor("outp", [S, D], BF, kind="ExternalOutput")

    EXP = mybir.ActivationFunctionType.Exp

    with tile.TileContext(nc) as tc:
        with (
            tc.tile_pool(name="wpool", bufs=1) as wpool,
            tc.tile_pool(name="spool", bufs=1) as spool,
            tc.tile_pool(name="apool", bufs=16) as apool,
            tc.tile_pool(name="rpool", bufs=4) as rpool,
            tc.tile_pool(name="bpool", bufs=4) as bpool,
            tc.tile_pool(name="opool", bufs=4) as opool,
            tc.tile_pool(name="ppair", bufs=2, space="PSUM") as ppair,
            tc.tile_pool(name="pmain", bufs=2, space="PSUM") as pmain,
            tc.tile_pool(name="pctx", bufs=2, space="PSUM") as pctx,
        ):
            # --- persistent tiles ---
            wqk8 = wpool.tile([128, 2, 2, NCC, 2, 128], F8)
            wv3 = wpool.tile([128, 3, NCC, 2, DG], F8)
            wo = wpool.tile([128, 2, D], BF)
            msk = wpool.tile([128, 4, QW], BF)
            xqk8 = spool.tile([128, NQ, 2, NCC, 2, QW], F8)
            xvv8 = spool.tile([128, NQ, 2, NCC, 2, QW], F8)
            # q/k score operands: [128 part(dh of t-half), t, ktile j, S];
            # j=1 is zero (DR pads contraction 64 -> 128).
            q8T = spool.tile([128, 2, 2, S], F8)
            k8T = spool.tile([128, 2, 2, S], F8)
            vv = spool.tile([128, S // 128, HPC, DH + 1], BF)
            ctxT = spool.tile([128, 2, S], BF)

            # column-phased input DMAs, ordered by first use: q-chunk 0's
            # scores, then its v-projection, then later phases.
            nc.sync.dma_start(wqk8[:, 0], wqkd.ap()[:, 0])
            nc.sync.dma_start(xqk8[:, 0], xqkd.ap()[:, 0])
            nc.sync.dma_start(wqk8[:, 1], wqkd.ap()[:, 1])
            nc.sync.dma_start(xqk8[:, 1], xqkd.ap()[:, 1])
            nc.sync.dma_start(msk[:], mskd.ap())
            nc.sync.dma_start(wv3[:], wv3d.ap())
            nc.sync.dma_start(xvv8[:, 0], xvvd.ap()[:, 0])
            nc.sync.dma_start(xvv8[:, 1], xvvd.ap()[:, 1])
            nc.sync.dma_start(wo[:], wod.ap())
            nc.sync.dma_start(xqk8[:, 2], xqkd.ap()[:, 2])
            nc.sync.dma_start(xvv8[:, 2], xvvd.ap()[:, 2])
            nc.sync.dma_start(xqk8[:, 3], xqkd.ap()[:, 3])
            nc.sync.dma_start(xvv8[:, 3], xvvd.ap()[:, 3])

            # zero the j=1 DR slots of q8T/k8T (u32-bitcast for packed memset)
            nc.vector.memset(q8T[:, :, 1, :], 0)
            nc.vector.memset(k8T[:, :, 1, :], 0)
            nc.vector.memset(vv[:, :, :, DH : DH + 1], 64.0)

            # --- q/k projections (direct fp8 DR) + fp8 lift ---
            # one (t, qk) tile; emitted just-in-time: si 0,1 upfront, si 2/3
            # hooked into attention on q-chunks 1/2 (after their DMA phase
            # lands) so the in-order PE stream never parks on a late phase.
            def emit_qkproj(si, t, qk):
                dst = q8T if qk == 0 else k8T
                ps = pmain.tile([128, QW], F32, tag="ps")
                off = 140 if si > 0 else 0
                with tc.high_priority(offset=off):
                    for cc in range(NCC):
                        nc.tensor.matmul(
                            ps[:],
                            wqk8[:, t, qk, cc, :, :],
                            xqk8[:, si, qk, cc, :, :],
                            start=(cc == 0),
                            stop=(cc == NCC - 1),
                            perf_mode=DR,
                        )
                with tc.high_priority(offset=off + 48):
                    nc.vector.tensor_scalar_mul(
                        dst[:, t, 0, si * QW : (si + 1) * QW],
                        ps[:],
                        1.0 / 64.0,
                    )

            for t in range(2):
                for qk in (0, 1):
                    emit_qkproj(0, t, qk)

            # --- v projection: 3-term compensated fp8 DR; vv holds 64*v ---
            def emit_vproj(st):
                sh, lc = st // 4, (st % 4) * 128
                ps = pmain.tile([128, DG], F32, tag="ps")
                for term in range(3):
                    sel = 1 if term == 2 else 0
                    for cc in range(NCC):
                        nc.tensor.matmul(
                            ps[:],
                            xvv8[:, sh, sel, cc, :, lc : lc + 128],
                            wv3[:, term, cc, :, :],
                            start=(term == 0 and cc == 0),
                            stop=(term == 2 and cc == NCC - 1),
                            perf_mode=DR,
                        )
                nc.vector.tensor_copy(
                    vv[:, st, :, 0:DH],
                    ps[:].rearrange("p (h e) -> p h e", e=DH),
                )

            # --- output projection (bf16) ---
            ob_tiles = {}

            def emit_wo_half(qt, nh, tail=False):
                if nh == 0:
                    ob = opool.tile([128, D], BF, tag="ob")
                    ob_tiles[qt] = ob
                ob = ob_tiles[qt]
                if tail and (2 * qt + nh) % 2:
                    # last q-chunk: attention PSUM pools are idle; borrow one
                    # so four Wo accumulations can be in flight
                    ops = pctx.tile([128, 512], F32, tag="cps")
                else:
                    ops = pmain.tile([128, 512], F32, tag="ps")
                for t in range(2):
                    nc.tensor.matmul(
                        ops[:],
                        ctxT[:, t, qt * 128 : (qt + 1) * 128],
                        wo[:, t, nh * 512 : (nh + 1) * 512],
                        start=(t == 0),
                        stop=(t == 1),
                    )
                if tail and qt % 2:
                    # split the PSUM lift between DVE and the now-idle ACT
                    nc.scalar.copy(ob[:, nh * 512 : (nh + 1) * 512], ops[:])
                else:
                    nc.vector.tensor_copy(
                        ob[:, nh * 512 : (nh + 1) * 512], ops[:]
                    )
                if nh == 1:
                    nc.sync.dma_start(
                        outp.ap()[qt * 128 : (qt + 1) * 128, :], ob[:]
                    )
                    del ob_tiles[qt]

            def emit_wo(qt, tail=False):
                emit_wo_half(qt, 0, tail)
                emit_wo_half(qt, 1, tail)

            # just-in-time projection work, spread between score pairs so the
            # in-order PE stream never parks the exp feed behind a projection
            # burst: each (qi, h, pc) slot runs at most one chunk, placed a
            # q-chunk ahead of its consumer where possible.
            # hook order matters: pmain pool slots recycle in EMISSION
            # order, so next-q-chunk score projections take the earliest
            # slots (their consumers unblock the exp stream) and the
            # v-projections come after (the at-pool buffers cover their
            # consumers' latency).
            hooks = {}
            hooks.update(
                {
                    (0, 0, 0): [lambda: emit_vproj(0), lambda: emit_vproj(1)],
                    (0, 0, 1): [lambda: emit_vproj(2), lambda: emit_vproj(3)],
                    (0, 2, 0): [lambda: emit_qkproj(1, 0, 0)],
                    (0, 2, 1): [lambda: emit_qkproj(1, 0, 1)],
                    (0, 3, 0): [lambda: emit_qkproj(1, 1, 0)],
                    (0, 3, 1): [lambda: emit_qkproj(1, 1, 1)],
                }
            )
            for pc in range(4):
                hooks[(1, 0, pc)] = [lambda st=4 + pc: emit_vproj(st)]
                hooks[(1, 1, pc)] = [
                    lambda t=pc // 2, qk=pc % 2: emit_qkproj(2, t, qk)
                ]
                hooks[(1, 2, pc)] = [lambda st=8 + pc: emit_vproj(st)]
                hooks[(2, 0, pc)] = [
                    lambda t=pc // 2, qk=pc % 2: emit_qkproj(3, t, qk)
                ]
                hooks[(2, 1, pc)] = [lambda st=12 + pc: emit_vproj(st)]

            # --- attention per q-chunk ---
            for qi in range(NQ):
                q_sl = slice(qi * QW, (qi + 1) * QW)
                nk = 4 * (qi + 1)
                for h in range(HPC):
                    t, p0 = h // 2, 64 * (h % 2)
                    cps = pctx.tile([DH + 1, QW], F32, tag="cps")
                    for pc in range(nk // 2):
                        d0 = 2 * pc - 4 * qi
                        c0p = max(0, 256 * pc - 512 * qi)
                        sps = ppair.tile([128, 2, QW], F32, tag="sps")
                        sc_off = None if qi == 0 else (96 if pc == 0 else 72)
                        with tc.high_priority(offset=sc_off):
                            for half in range(2):
                                kc = 2 * pc + half
                                nc.tensor.matmul(
                                    sps[:, half, c0p:QW],
                                    k8T[p0 : p0 + 64, t, :, kc * 128 : (kc + 1) * 128],
                                    q8T[p0 : p0 + 64, t, :, qi * QW + c0p : (qi + 1) * QW],
                                    start=True,
                                    stop=True,
                                    perf_mode=DR,
                                )
                        at = apool.tile([128, 2, QW], BF, tag="at")
                        with tc.high_priority(offset=sc_off):
                            nc.scalar.activation(
                                at[:, :, c0p:QW], sps[:, :, c0p:QW], EXP
                            )
                        if d0 >= 0:
                            m1 = min(128 * d0 + 256, QW)
                            nc.vector.tensor_mul(
                                at[:, :, c0p:m1],
                                at[:, :, c0p:m1],
                                msk[:, d0 : d0 + 2, c0p:m1],
                            )
                        for fn in hooks.get((qi, h, pc), ()):
                            fn()
                        if qi > 0 and h >= 1 and pc in (1, 2):
                            emit_wo_half((qi - 1) * 4 + h - 1, pc - 1)
                        for half in range(2):
                            kc = 2 * pc + half
                            c0 = max(0, 128 * kc - 512 * qi)
                            nc.tensor.matmul(
                                cps[:, c0:QW],
                                vv[:, kc, h, :],
                                at[:, half, c0:QW],
                                start=(kc == 0),
                                stop=(kc == nk - 1),
                            )
                    # softmax normalization: ctxT = (64 ctx) * (1/(64 den)),
                    # 1/den broadcast across the 64 dh partitions on GPSIMD
                    # (tensor_tensor cannot take two PSUM operands).
                    rc = rpool.tile([1, QW], F32R)
                    with nc.allow_low_precision(reason="f32r bits ~ f32"):
                        nc.vector.reciprocal(rc[:], cps[DH : DH + 1, :])
                    rc64 = bpool.tile([64, QW], F32R)
                    nc.gpsimd.partition_broadcast(rc64[:], rc[:], channels=64)
                    nc.vector.tensor_mul(
                        ctxT[p0 : p0 + 64, t, q_sl], cps[0:DH, :], rc64[:]
                    )
                if qi > 0:
                    emit_wo(qi * 4 - 1)
            for j in range(4):
                emit_wo(12 + j, tail=True)


def build_program():
    if "nc" in _CACHE:
        return _CACHE["nc"]
    nc = bacc.Bacc(
        "TRN2", target_bir_lowering=False, debug=False, num_devices=N_CORES
    )
    _emit(nc)
    nc.compile()
    _CACHE["nc"] = nc
    return nc


def _pack_dr(a):
    """[D, N] -> [128, D//256, 2, N] with D-index = cc*256 + j*128 + p."""
    n = a.shape[1]
    return np.ascontiguousarray(
        a.reshape(D // 256, 2, 128, n).transpose(2, 0, 1, 3)
    )


def _pack_x(a):
    """[D, S] -> [128, NQ, NCC, 2, QW]: DR pack + 512-col phase outermost."""
    return np.ascontiguousarray(
        _pack_dr(a).reshape(128, NCC, 2, NQ, QW).transpose(0, 3, 1, 2, 4)
    )


def _prep_in_maps(query, key, value, Wq, Wk, Wv, Wo):
    bf = ml_dtypes.bfloat16
    f8 = ml_dtypes.float8_e4m3

    p, i, j = np.ogrid[0:128, 0:4, 0:QW]
    msk = (j >= 128 * i + p).astype(bf)

    xT = {}
    for b in range(B):
        x8q = np.ascontiguousarray(query[b].T).astype(f8)
        x8k = np.ascontiguousarray(key[b].T).astype(f8)
        xT[("qk", b)] = np.ascontiguousarray(
            np.stack([_pack_x(x8q), _pack_x(x8k)], axis=2)
        )
        tv = np.ascontiguousarray(value[b].T)
        x8v = tv.astype(f8)
        dx16 = ((tv - x8v.astype(np.float32)) * 16.0).astype(f8)
        xT[("vv", b)] = np.ascontiguousarray(
            np.stack([_pack_x(x8v), _pack_x(dx16)], axis=2)
        )

    in_maps = []
    for c in range(N_CORES):
        b, g = c // HPC, c % HPC
        rows = slice(g * DG, (g + 1) * DG)
        wqT = np.ascontiguousarray(Wq[rows].T) * 16.0  # [D, DG]
        wkT = np.ascontiguousarray(Wk[rows].T) * 32.0
        wvT = np.ascontiguousarray(Wv[rows].T) * 64.0
        wv8 = wvT.astype(f8)
        dwv8 = (wvT - wv8.astype(np.float32)).astype(f8)
        wv4 = (wvT / 16.0).astype(f8)
        in_maps.append(
            {
                "xqk8": xT[("qk", b)],
                "xvv8": xT[("vv", b)],
                "wqk8": np.ascontiguousarray(
                    np.stack(
                        [_pack_dr(wqT.astype(f8)), _pack_dr(wkT.astype(f8))],
                        axis=1,
                    )  # [128, qk, cc, j, DG]
                    .reshape(128, 2, NCC, 2, 2, 128)
                    .transpose(0, 4, 1, 2, 3, 5)  # [128, t, qk, cc, j, 128]
                ),
                "wv3": np.ascontiguousarray(
                    np.stack(
                        [_pack_dr(wv8), _pack_dr(dwv8), _pack_dr(wv4)], axis=1
                    )
                ),
                "woT": np.ascontiguousarray(Wo[:, rows].T)
                .astype(bf)
                .reshape(2, 128, D)
                .transpose(1, 0, 2)
                .copy(),
                "msk": msk,
            }
        )
    return in_maps


def kernel(query, key, value, Wq, Wk, Wv, Wo):
    query = np.asarray(query, dtype=np.float32)
    key = np.asarray(key, dtype=np.float32)
    value = np.asarray(value, dtype=np.float32)
    Wq = np.asarray(Wq, dtype=np.float32)
    Wk = np.asarray(Wk, dtype=np.float32)
    Wv = np.asarray(Wv, dtype=np.float32)
    Wo = np.asarray(Wo, dtype=np.float32)

    nc = build_program()
    in_maps = _prep_in_maps(query, key, value, Wq, Wk, Wv, Wo)
    res = run_bass_kernel_spmd(
        nc, in_maps, core_ids=list(range(N_CORES)), trace=False
    )
    out = np.zeros((B, S, D), dtype=np.float32)
    for b in range(B):
        for g in range(HPC):
            out[b] += res.results[b * HPC + g]["outp"].astype(np.float32)
    return out


# revision 7
# speedup vs baseline: 1.3548x; 1.0005x over previous
"""Multi-head causal attention (B=2, S=2048, D=1024, H=16, DH=64) on 8 NeuronCores.

Sharding: data-parallel over batch (2) x tensor-parallel over heads (4 groups
of 4 heads). Core c handles batch c//4, heads 4*(c%4)..4*(c%4)+3. Host sums
the 4 Wo row-shard partials per batch.

Precision/engine plan (validated numerically against the reference):
- Q/K projections: direct fp8(e4m3) DoubleRow matmuls (K=256/instr). Host
  uploads x8 = fp8(x) and Wq*16 / Wk*32 in fp8; PSUM holds 16*q / 32*k and the
  DVE lift scales by 1/64 into fp8 score operands q/4 and k/2, so the q*k
  matmul directly yields q*k/8 (the softmax scale).
- Scores: fp8 DoubleRow, contraction dh=64 on a 64-partition slice with the
  second DR k-tile zeroed.
- V projection: 3-term compensated fp8 DoubleRow (x8@W64 + x8@dW + 16dx@4W)
  keeping v at ~bf16 accuracy; PSUM holds 64*v which cancels against the
  1/(64*den) reciprocal in the softmax normalization.
- attention*V, Wo: bf16 (fp8 here fails the error budget).
- exp on ACT; causal mask multiply on GPSIMD; 1/den broadcast by rank-1 PE
  matmul; PSUM->SBUF lifts and output copies on DVE.

Dataflow choices for pipeline fill: q/k (and v/dv) inputs ship as single
combined dram tensors DMA'd in column phases (cols 0:512 first) so the first
score pairs are ready ~5us in; projections emit si-major (q then k per si) so
attention on q-chunk 0 unblocks after 4 of 16 projection tiles; v-projection
chunks are spread inside the first head's pair loop of each q-chunk so the
ACT exp stream never starves behind a projection burst.
"""

import numpy as np
import ml_dtypes

import concourse.bass as bass  # noqa: F401
import concourse.mybir as mybir
import concourse.tile as tile
from concourse import bacc
from concourse.bass_utils import run_bass_kernel_spmd

B, S, D, H, DH = 2, 2048, 1024, 16, 64
N_CORES = 8
HPC = 4            # heads per core
DG = HPC * DH      # 256 head dims per core
QW = 512           # q-chunk width
NQ = S // QW       # 4 q-chunks
NCC = D // 256     # 4 DR contraction chunks for projections

BF = mybir.dt.bfloat16
F8 = mybir.dt.float8e4
F32 = mybir.dt.float32
F32R = mybir.dt.float32r
DR = mybir.MatmulPerfMode.DoubleRow

_CACHE = {}


def _emit(nc):
    # x layouts put the 512-wide column phase OUTERMOST so each phase DMA is
    # one contiguous per-partition subregion (cheap exact dependency ranges).
    xqkd = nc.dram_tensor("xqk8", [128, NQ, 2, NCC, 2, QW], F8, kind="ExternalInput")
    xvvd = nc.dram_tensor("xvv8", [128, NQ, 2, NCC, 2, QW], F8, kind="ExternalInput")
    wqkd = nc.dram_tensor("wqk8", [128, 2, 2, NCC, 2, 128], F8, kind="ExternalInput")
    wv3d = nc.dram_tensor("wv3", [128, 3, NCC, 2, DG], F8, kind="ExternalInput")
    wod = nc.dram_tensor("woT", [128, 2, D], BF, kind="ExternalInput")
    mskd = nc.dram_tensor("msk", [128, 4, QW], BF, kind="ExternalInput")
    outp = nc.dram_tensor("outp", [S, D], BF, kind="ExternalOutput")

    EXP = mybir.ActivationFunctionType.Exp

    with tile.TileContext(nc) as tc:
        with (
            tc.tile_pool(name="wpool", bufs=1) as wpool,
            tc.tile_pool(name="spool", bufs=1) as spool,
            tc.tile_pool(name="apool", bufs=16) as apool,
            tc.tile_pool(name="rpool", bufs=4) as rpool,
            tc.tile_pool(name="bpool", bufs=4) as bpool,
            tc.tile_pool(name="opool", bufs=4) as opool,
            tc.tile_pool(name="ppair", bufs=2, space="PSUM") as ppair,
            tc.tile_pool(name="pmain", bufs=2, space="PSUM") as pmain,
            tc.tile_pool(name="pctx", bufs=2, space="PSUM") as pctx,
        ):
            # --- persistent tiles ---
            wqk8 = wpool.tile([128, 2, 2, NCC, 2, 128], F8)
            wv3 = wpool.tile([128, 3, NCC, 2, DG], F8)
            wo = wpool.tile([128, 2, D], BF)
            msk = wpool.tile([128, 4, QW], BF)
            xqk8 = spool.tile([128, NQ, 2, NCC, 2, QW], F8)
            xvv8 = spool.tile([128, NQ, 2, NCC, 2, QW], F8)
            # q/k score operands: [128 part(dh of t-half), t, ktile j, S];
            # j=1 is zero (DR pads contraction 64 -> 128).
            q8T = spool.tile([128, 2, 2, S], F8)
            k8T = spool.tile([128, 2, 2, S], F8)
            vv = spool.tile([128, S // 128, HPC, DH + 1], BF)
            ctxT = spool.tile([128, 2, S], BF)

            # column-phased input DMAs, ordered by first use: q-chunk 0's
            # scores, then its v-projection, then later phases.
            nc.sync.dma_start(wqk8[:, 0], wqkd.ap()[:, 0])
            nc.sync.dma_start(xqk8[:, 0, 0], xqkd.ap()[:, 0, 0])
            nc.sync.dma_start(xqk8[:, 0, 1], xqkd.ap()[:, 0, 1])
            nc.sync.dma_start(wqk8[:, 1], wqkd.ap()[:, 1])
            nc.sync.dma_start(xqk8[:, 1], xqkd.ap()[:, 1])
            nc.sync.dma_start(msk[:], mskd.ap())
            nc.sync.dma_start(wv3[:], wv3d.ap())
            nc.sync.dma_start(xvv8[:, 0], xvvd.ap()[:, 0])
            nc.sync.dma_start(xvv8[:, 1], xvvd.ap()[:, 1])
            nc.sync.dma_start(wo[:], wod.ap())
            nc.sync.dma_start(xqk8[:, 2], xqkd.ap()[:, 2])
            nc.sync.dma_start(xvv8[:, 2], xvvd.ap()[:, 2])
            nc.sync.dma_start(xqk8[:, 3], xqkd.ap()[:, 3])
            nc.sync.dma_start(xvv8[:, 3], xvvd.ap()[:, 3])

            # zero the j=1 DR slots of q8T/k8T (u32-bitcast for packed memset)
            nc.vector.memset(q8T[:, :, 1, :], 0)
            nc.vector.memset(k8T[:, :, 1, :], 0)
            nc.vector.memset(vv[:, :, :, DH : DH + 1], 64.0)

            # --- q/k projections (direct fp8 DR) + fp8 lift ---
            # one (t, qk) tile; emitted just-in-time: si 0,1 upfront, si 2/3
            # hooked into attention on q-chunks 1/2 (after their DMA phase
            # lands) so the in-order PE stream never parks on a late phase.
            def emit_qkproj(si, t, qk):
                dst = q8T if qk == 0 else k8T
                ps = pmain.tile([128, QW], F32, tag="ps")
                off = 140 if si > 0 else 0
                with tc.high_priority(offset=off):
                    for cc in range(NCC):
                        nc.tensor.matmul(
                            ps[:],
                            wqk8[:, t, qk, cc, :, :],
                            xqk8[:, si, qk, cc, :, :],
                            start=(cc == 0),
                            stop=(cc == NCC - 1),
                            perf_mode=DR,
                        )
                with tc.high_priority(offset=off + 48):
                    nc.vector.tensor_scalar_mul(
                        dst[:, t, 0, si * QW : (si + 1) * QW],
                        ps[:],
                        1.0 / 64.0,
                    )

            for t in range(2):
                for qk in (0, 1):
                    emit_qkproj(0, t, qk)

            # --- v projection: 3-term compensated fp8 DR; vv holds 64*v ---
            def emit_vproj(st):
                sh, lc = st // 4, (st % 4) * 128
                ps = pmain.tile([128, DG], F32, tag="ps")
                for term in range(3):
                    sel = 1 if term == 2 else 0
                    for cc in range(NCC):
                        nc.tensor.matmul(
                            ps[:],
                            xvv8[:, sh, sel, cc, :, lc : lc + 128],
                            wv3[:, term, cc, :, :],
                            start=(term == 0 and cc == 0),
                            stop=(term == 2 and cc == NCC - 1),
                            perf_mode=DR,
                        )
                nc.vector.tensor_copy(
                    vv[:, st, :, 0:DH],
                    ps[:].rearrange("p (h e) -> p h e", e=DH),
                )

            # --- output projection (bf16) ---
            ob_tiles = {}

            def emit_wo_half(qt, nh, tail=False):
                if nh == 0:
                    ob = opool.tile([128, D], BF, tag="ob")
                    ob_tiles[qt] = ob
                ob = ob_tiles[qt]
                if tail and (2 * qt + nh) % 2:
                    # last q-chunk: attention PSUM pools are idle; borrow one
                    # so four Wo accumulations can be in flight
                    ops = pctx.tile([128, 512], F32, tag="cps")
                else:
                    ops = pmain.tile([128, 512], F32, tag="ps")
                for t in range(2):
                    nc.tensor.matmul(
                        ops[:],
                        ctxT[:, t, qt * 128 : (qt + 1) * 128],
                        wo[:, t, nh * 512 : (nh + 1) * 512],
                        start=(t == 0),
                        stop=(t == 1),
                    )
                if tail and qt % 2:
                    # split the PSUM lift between DVE and the now-idle ACT
                    nc.scalar.copy(ob[:, nh * 512 : (nh + 1) * 512], ops[:])
                else:
                    nc.vector.tensor_copy(
                        ob[:, nh * 512 : (nh + 1) * 512], ops[:]
                    )
                if nh == 1:
                    nc.sync.dma_start(
                        outp.ap()[qt * 128 : (qt + 1) * 128, :], ob[:]
                    )
                    del ob_tiles[qt]

            def emit_wo(qt, tail=False):
                emit_wo_half(qt, 0, tail)
                emit_wo_half(qt, 1, tail)

            # just-in-time projection work, spread between score pairs so the
            # in-order PE stream never parks the exp feed behind a projection
            # burst: each (qi, h, pc) slot runs at most one chunk, placed a
            # q-chunk ahead of its consumer where possible.
            # hook order matters: pmain pool slots recycle in EMISSION
            # order, so next-q-chunk score projections take the earliest
            # slots (their consumers unblock the exp stream) and the
            # v-projections come after (the at-pool buffers cover their
            # consumers' latency).
            hooks = {}
            hooks.update(
                {
                    (0, 0, 0): [lambda: emit_vproj(0), lambda: emit_vproj(1)],
                    (0, 0, 1): [lambda: emit_vproj(2), lambda: emit_vproj(3)],
                    (0, 1, 0): [lambda: emit_qkproj(1, 0, 0)],
                    (0, 1, 1): [lambda: emit_qkproj(1, 0, 1)],
                    (0, 2, 0): [lambda: emit_qkproj(1, 1, 0)],
                    (0, 2, 1): [lambda: emit_qkproj(1, 1, 1)],
                }
            )
            for pc in range(4):
                hooks[(1, 0, pc)] = [lambda st=4 + pc: emit_vproj(st)]
                hooks[(1, 1, pc)] = [
                    lambda t=pc // 2, qk=pc % 2: emit_qkproj(2, t, qk)
                ]
                hooks[(1, 2, pc)] = [lambda st=8 + pc: emit_vproj(st)]
                hooks[(2, 0, pc)] = [
                    lambda t=pc // 2, qk=pc % 2: emit_qkproj(3, t, qk)
                ]
                hooks[(2, 1, pc)] = [lambda st=12 + pc: emit_vproj(st)]

            # --- attention per q-chunk ---
            for qi in range(NQ):
                q_sl = slice(qi * QW, (qi + 1) * QW)
                nk = 4 * (qi + 1)
                for h in range(HPC):
                    t, p0 = h // 2, 64 * (h % 2)
                    cps = pctx.tile([DH + 1, QW], F32, tag="cps")
                    for pc in range(nk // 2):
                        d0 = 2 * pc - 4 * qi
                        c0p = max(0, 256 * pc - 512 * qi)
                        sps = ppair.tile([128, 2, QW], F32, tag="sps")
                        sc_off = None if qi == 0 else (96 if pc == 0 else 72)
                        with tc.high_priority(offset=sc_off):
                            for half in range(2):
                                kc = 2 * pc + half
                                nc.tensor.matmul(
                                    sps[:, half, c0p:QW],
                                    k8T[p0 : p0 + 64, t, :, kc * 128 : (kc + 1) * 128],
                                    q8T[p0 : p0 + 64, t, :, qi * QW + c0p : (qi + 1) * QW],
                                    start=True,
                                    stop=True,
                                    perf_mode=DR,
                                )
                        at = apool.tile([128, 2, QW], BF, tag="at")
                        with tc.high_priority(offset=sc_off):
                            nc.scalar.activation(
                                at[:, :, c0p:QW], sps[:, :, c0p:QW], EXP
                            )
                        if d0 >= 0:
                            m1 = min(128 * d0 + 256, QW)
                            nc.vector.tensor_mul(
                                at[:, :, c0p:m1],
                                at[:, :, c0p:m1],
                                msk[:, d0 : d0 + 2, c0p:m1],
                            )
                        for fn in hooks.get((qi, h, pc), ()):
                            fn()
                        if qi > 0 and h >= 1 and pc in (1, 2):
                            emit_wo_half((qi - 1) * 4 + h - 1, pc - 1)
                        for half in range(2):
                            kc = 2 * pc + half
                            c0 = max(0, 128 * kc - 512 * qi)
                            nc.tensor.matmul(
                                cps[:, c0:QW],
                                vv[:, kc, h, :],
                                at[:, half, c0:QW],
                                start=(kc == 0),
                                stop=(kc == nk - 1),
                            )
                    # softmax normalization: ctxT = (64 ctx) * (1/(64 den)),
                    # 1/den broadcast across the 64 dh partitions on GPSIMD
                    # (tensor_tensor cannot take two PSUM operands).
                    nrm_off = 64 if qi == NQ - 1 else 0
                    with tc.high_priority(offset=nrm_off):
                        rc = rpool.tile([1, QW], F32R)
                        with nc.allow_low_precision(reason="f32r bits ~ f32"):
                            nc.vector.reciprocal(rc[:], cps[DH : DH + 1, :])
                        rc64 = bpool.tile([64, QW], F32R)
                        nc.gpsimd.partition_broadcast(rc64[:], rc[:], channels=64)
                        nc.vector.tensor_mul(
                            ctxT[p0 : p0 + 64, t, q_sl], cps[0:DH, :], rc64[:]
                        )
                if qi > 0:
                    emit_wo(qi * 4 - 1)
            for j in range(4):
                emit_wo(12 + j, tail=True)


def build_program():
    if "nc" in _CACHE:
        return _CACHE["nc"]
    nc = bacc.Bacc(
        "TRN2", target_bir_lowering=False, debug=False, num_devices=N_CORES
    )
    _emit(nc)
    nc.compile()
    _CACHE["nc"] = nc
    return nc


def _pack_dr(a):
    """[D, N] -> [128, D//256, 2, N] with D-index = cc*256 + j*128 + p."""
    n = a.shape[1]
    return np.ascontiguousarray(
        a.reshape(D // 256, 2, 128, n).transpose(2, 0, 1, 3)
    )


def _pack_x(a):
    """[D, S] -> [128, NQ, NCC, 2, QW]: DR pack + 512-col phase outermost."""
    return np.ascontiguousarray(
        _pack_dr(a).reshape(128, NCC, 2, NQ, QW).transpose(0, 3, 1, 2, 4)
    )


def _prep_in_maps(query, key, value, Wq, Wk, Wv, Wo):
    bf = ml_dtypes.bfloat16
    f8 = ml_dtypes.float8_e4m3

    p, i, j = np.ogrid[0:128, 0:4, 0:QW]
    msk = (j >= 128 * i + p).astype(bf)

    xT = {}
    for b in range(B):
        x8q = np.ascontiguousarray(query[b].T).astype(f8)
        x8k = np.ascontiguousarray(key[b].T).astype(f8)
        xT[("qk", b)] = np.ascontiguousarray(
            np.stack([_pack_x(x8q), _pack_x(x8k)], axis=2)
        )
        tv = np.ascontiguousarray(value[b].T)
        x8v = tv.astype(f8)
        dx16 = ((tv - x8v.astype(np.float32)) * 16.0).astype(f8)
        xT[("vv", b)] = np.ascontiguousarray(
            np.stack([_pack_x(x8v), _pack_x(dx16)], axis=2)
        )

    in_maps = []
    for c in range(N_CORES):
        b, g = c // HPC, c % HPC
        rows = slice(g * DG, (g + 1) * DG)
        wqT = np.ascontiguousarray(Wq[rows].T) * 16.0  # [D, DG]
        wkT = np.ascontiguousarray(Wk[rows].T) * 32.0
        wvT = np.ascontiguousarray(Wv[rows].T) * 64.0
        wv8 = wvT.astype(f8)
        dwv8 = (wvT - wv8.astype(np.float32)).astype(f8)
        wv4 = (wvT / 16.0).astype(f8)
        in_maps.append(
            {
                "xqk8": xT[("qk", b)],
                "xvv8": xT[("vv", b)],
                "wqk8": np.ascontiguousarray(
                    np.stack(
                        [_pack_dr(wqT.astype(f8)), _pack_dr(wkT.astype(f8))],
                        axis=1,
                    )  # [128, qk, cc, j, DG]
                    .reshape(128, 2, NCC, 2, 2, 128)
                    .transpose(0, 4, 1, 2, 3, 5)  # [128, t, qk, cc, j, 128]
                ),
                "wv3": np.ascontiguousarray(
                    np.stack(
                        [_pack_dr(wv8), _pack_dr(dwv8), _pack_dr(wv4)], axis=1
                    )
                ),
                "woT": np.ascontiguousarray(Wo[:, rows].T)
                .astype(bf)
                .reshape(2, 128, D)
                .transpose(1, 0, 2)
                .copy(),
                "msk": msk,
            }
        )
    return in_maps


def kernel(query, key, value, Wq, Wk, Wv, Wo):
    query = np.asarray(query, dtype=np.float32)
    key = np.asarray(key, dtype=np.float32)
    value = np.asarray(value, dtype=np.float32)
    Wq = np.asarray(Wq, dtype=np.float32)
    Wk = np.asarray(Wk, dtype=np.float32)
    Wv = np.asarray(Wv, dtype=np.float32)
    Wo = np.asarray(Wo, dtype=np.float32)

    nc = build_program()
    in_maps = _prep_in_maps(query, key, value, Wq, Wk, Wv, Wo)
    res = run_bass_kernel_spmd(
        nc, in_maps, core_ids=list(range(N_CORES)), trace=False
    )
    out = np.zeros((B, S, D), dtype=np.float32)
    for b in range(B):
        for g in range(HPC):
            out[b] += res.results[b * HPC + g]["outp"].astype(np.float32)
    return out


# revision 8
# speedup vs baseline: 1.3571x; 1.0017x over previous
"""Multi-head causal attention (B=2, S=2048, D=1024, H=16, DH=64) on 8 NeuronCores.

Sharding: data-parallel over batch (2) x tensor-parallel over heads (4 groups
of 4 heads). Core c handles batch c//4, heads 4*(c%4)..4*(c%4)+3. Host sums
the 4 Wo row-shard partials per batch.

Precision/engine plan (validated numerically against the reference):
- Q/K projections: direct fp8(e4m3) DoubleRow matmuls (K=256/instr). Host
  uploads x8 = fp8(x) and Wq*16 / Wk*32 in fp8; PSUM holds 16*q / 32*k and the
  DVE lift scales by 1/64 into fp8 score operands q/4 and k/2, so the q*k
  matmul directly yields q*k/8 (the softmax scale).
- Scores: fp8 DoubleRow, contraction dh=64 on a 64-partition slice with the
  second DR k-tile zeroed.
- V projection: 3-term compensated fp8 DoubleRow (x8@W64 + x8@dW + 16dx@4W)
  keeping v at ~bf16 accuracy; PSUM holds 64*v which cancels against the
  1/(64*den) reciprocal in the softmax normalization.
- attention*V, Wo: bf16 (fp8 here fails the error budget).
- exp on ACT; causal mask multiply on GPSIMD; 1/den broadcast by rank-1 PE
  matmul; PSUM->SBUF lifts and output copies on DVE.

Dataflow choices for pipeline fill: q/k (and v/dv) inputs ship as single
combined dram tensors DMA'd in column phases (cols 0:512 first) so the first
score pairs are ready ~5us in; projections emit si-major (q then k per si) so
attention on q-chunk 0 unblocks after 4 of 16 projection tiles; v-projection
chunks are spread inside the first head's pair loop of each q-chunk so the
ACT exp stream never starves behind a projection burst.
"""

import numpy as np
import ml_dtypes

import concourse.bass as bass  # noqa: F401
import concourse.mybir as mybir
import concourse.tile as tile
from concourse import bacc
from concourse.bass_utils import run_bass_kernel_spmd

B, S, D, H, DH = 2, 2048, 1024, 16, 64
N_CORES = 8
HPC = 4            # heads per core
DG = HPC * DH      # 256 head dims per core
QW = 512           # q-chunk width
NQ = S // QW       # 4 q-chunks
NCC = D // 256     # 4 DR contraction chunks for projections

BF = mybir.dt.bfloat16
F8 = mybir.dt.float8e4
F32 = mybir.dt.float32
F32R = mybir.dt.float32r
DR = mybir.MatmulPerfMode.DoubleRow

_CACHE = {}


def _emit(nc):
    # x layouts put the 512-wide column phase OUTERMOST so each phase DMA is
    # one contiguous per-partition subregion (cheap exact dependency ranges).
    xqkd = nc.dram_tensor("xqk8", [128, NQ, 2, NCC, 2, QW], F8, kind="ExternalInput")
    xvvd = nc.dram_tensor("xvv8", [128, NQ, 2, NCC, 2, QW], F8, kind="ExternalInput")
    wqkd = nc.dram_tensor("wqk8", [128, 2, 2, NCC, 2, 128], F8, kind="ExternalInput")
    wv3d = nc.dram_tensor("wv3", [128, 3, NCC, 2, DG], F8, kind="ExternalInput")
    wod = nc.dram_tensor("woT", [128, 2, D], BF, kind="ExternalInput")
    mskd = nc.dram_tensor("msk", [128, 4, QW], BF, kind="ExternalInput")
    outp = nc.dram_tensor("outp", [S, D], BF, kind="ExternalOutput")

    EXP = mybir.ActivationFunctionType.Exp

    with tile.TileContext(nc) as tc:
        with (
            tc.tile_pool(name="wpool", bufs=1) as wpool,
            tc.tile_pool(name="spool", bufs=1) as spool,
            tc.tile_pool(name="apool", bufs=20) as apool,
            tc.tile_pool(name="rpool", bufs=6) as rpool,
            tc.tile_pool(name="bpool", bufs=6) as bpool,
            tc.tile_pool(name="opool", bufs=6) as opool,
            tc.tile_pool(name="ppair", bufs=2, space="PSUM") as ppair,
            tc.tile_pool(name="pmain", bufs=2, space="PSUM") as pmain,
            tc.tile_pool(name="pctx", bufs=2, space="PSUM") as pctx,
        ):
            # --- persistent tiles ---
            wqk8 = wpool.tile([128, 2, 2, NCC, 2, 128], F8)
            wv3 = wpool.tile([128, 3, NCC, 2, DG], F8)
            wo = wpool.tile([128, 2, D], BF)
            msk = wpool.tile([128, 4, QW], BF)
            xqk8 = spool.tile([128, NQ, 2, NCC, 2, QW], F8)
            xvv8 = spool.tile([128, NQ, 2, NCC, 2, QW], F8)
            # q/k score operands: [128 part(dh of t-half), t, ktile j, S];
            # j=1 is zero (DR pads contraction 64 -> 128).
            q8T = spool.tile([128, 2, 2, S], F8)
            k8T = spool.tile([128, 2, 2, S], F8)
            vv = spool.tile([128, S // 128, HPC, DH + 1], BF)
            ctxT = spool.tile([128, 2, S], BF)

            # column-phased input DMAs, ordered by first use: q-chunk 0's
            # scores, then its v-projection, then later phases.
            nc.sync.dma_start(wqk8[:, 0], wqkd.ap()[:, 0])
            nc.sync.dma_start(xqk8[:, 0, 0], xqkd.ap()[:, 0, 0])
            nc.sync.dma_start(xqk8[:, 0, 1], xqkd.ap()[:, 0, 1])
            nc.sync.dma_start(wqk8[:, 1], wqkd.ap()[:, 1])
            nc.sync.dma_start(xqk8[:, 1], xqkd.ap()[:, 1])
            nc.sync.dma_start(msk[:], mskd.ap())
            nc.sync.dma_start(wv3[:], wv3d.ap())
            nc.sync.dma_start(xvv8[:, 0], xvvd.ap()[:, 0])
            nc.sync.dma_start(xvv8[:, 1], xvvd.ap()[:, 1])
            nc.sync.dma_start(wo[:], wod.ap())
            nc.sync.dma_start(xqk8[:, 2], xqkd.ap()[:, 2])
            nc.sync.dma_start(xvv8[:, 2], xvvd.ap()[:, 2])
            nc.sync.dma_start(xqk8[:, 3], xqkd.ap()[:, 3])
            nc.sync.dma_start(xvv8[:, 3], xvvd.ap()[:, 3])

            # zero the j=1 DR slots of q8T/k8T (u32-bitcast for packed memset)
            nc.gpsimd.memset(q8T[:, :, 1, :].bitcast(mybir.dt.uint32), 0)
            nc.gpsimd.memset(k8T[:, :, 1, :].bitcast(mybir.dt.uint32), 0)
            nc.gpsimd.memset(vv[:, :, :, DH : DH + 1], 64.0)

            # --- q/k projections (direct fp8 DR) + fp8 lift ---
            # one (t, qk) tile; emitted just-in-time: si 0,1 upfront, si 2/3
            # hooked into attention on q-chunks 1/2 (after their DMA phase
            # lands) so the in-order PE stream never parks on a late phase.
            def emit_qkproj(si, t, qk):
                dst = q8T if qk == 0 else k8T
                ps = pmain.tile([128, QW], F32, tag="ps")
                off = 140 if si > 0 else 0
                with tc.high_priority(offset=off):
                    for cc in range(NCC):
                        nc.tensor.matmul(
                            ps[:],
                            wqk8[:, t, qk, cc, :, :],
                            xqk8[:, si, qk, cc, :, :],
                            start=(cc == 0),
                            stop=(cc == NCC - 1),
                            perf_mode=DR,
                        )
                with tc.high_priority(offset=off + 48):
                    nc.vector.tensor_scalar_mul(
                        dst[:, t, 0, si * QW : (si + 1) * QW],
                        ps[:],
                        1.0 / 64.0,
                    )

            for t in range(2):
                for qk in (0, 1):
                    emit_qkproj(0, t, qk)

            # --- v projection: 3-term compensated fp8 DR; vv holds 64*v ---
            def emit_vproj(st):
                sh, lc = st // 4, (st % 4) * 128
                ps = pmain.tile([128, DG], F32, tag="ps")
                for term in range(3):
                    sel = 1 if term == 2 else 0
                    for cc in range(NCC):
                        nc.tensor.matmul(
                            ps[:],
                            xvv8[:, sh, sel, cc, :, lc : lc + 128],
                            wv3[:, term, cc, :, :],
                            start=(term == 0 and cc == 0),
                            stop=(term == 2 and cc == NCC - 1),
                            perf_mode=DR,
                        )
                nc.vector.tensor_copy(
                    vv[:, st, :, 0:DH],
                    ps[:].rearrange("p (h e) -> p h e", e=DH),
                )

            # --- output projection (bf16) ---
            ob_tiles = {}

            def emit_wo_half(qt, nh, tail=False):
                if nh == 0:
                    ob = opool.tile([128, D], BF, tag="ob")
                    ob_tiles[qt] = ob
                ob = ob_tiles[qt]
                if tail and (2 * qt + nh) % 2:
                    # last q-chunk: attention PSUM pools are idle; borrow one
                    # so four Wo accumulations can be in flight
                    ops = pctx.tile([128, 512], F32, tag="cps")
                else:
                    ops = pmain.tile([128, 512], F32, tag="ps")
                for t in range(2):
                    nc.tensor.matmul(
                        ops[:],
                        ctxT[:, t, qt * 128 : (qt + 1) * 128],
                        wo[:, t, nh * 512 : (nh + 1) * 512],
                        start=(t == 0),
                        stop=(t == 1),
                    )
                if tail and qt % 2:
                    # split the PSUM lift between DVE and the now-idle ACT
                    nc.scalar.copy(ob[:, nh * 512 : (nh + 1) * 512], ops[:])
                else:
                    nc.vector.tensor_copy(
                        ob[:, nh * 512 : (nh + 1) * 512], ops[:]
                    )
                if nh == 1:
                    nc.sync.dma_start(
                        outp.ap()[qt * 128 : (qt + 1) * 128, :], ob[:]
                    )
                    del ob_tiles[qt]

            def emit_wo(qt, tail=False):
                emit_wo_half(qt, 0, tail)
                emit_wo_half(qt, 1, tail)

            # just-in-time projection work, spread between score pairs so the
            # in-order PE stream never parks the exp feed behind a projection
            # burst: each (qi, h, pc) slot runs at most one chunk, placed a
            # q-chunk ahead of its consumer where possible.
            # hook order matters: pmain pool slots recycle in EMISSION
            # order, so next-q-chunk score projections take the earliest
            # slots (their consumers unblock the exp stream) and the
            # v-projections come after (the at-pool buffers cover their
            # consumers' latency).
            hooks = {}
            hooks.update(
                {
                    (0, 0, 0): [lambda: emit_vproj(0), lambda: emit_vproj(1)],
                    (0, 0, 1): [lambda: emit_vproj(2), lambda: emit_vproj(3)],
                    (0, 1, 0): [lambda: emit_qkproj(1, 0, 0)],
                    (0, 1, 1): [lambda: emit_qkproj(1, 0, 1)],
                    (0, 2, 0): [lambda: emit_qkproj(1, 1, 0)],
                    (0, 2, 1): [lambda: emit_qkproj(1, 1, 1)],
                }
            )
            for pc in range(4):
                hooks[(1, 0, pc)] = [lambda st=4 + pc: emit_vproj(st)]
                hooks[(1, 1, pc)] = [
                    lambda t=pc // 2, qk=pc % 2: emit_qkproj(2, t, qk)
                ]
                hooks[(1, 2, pc)] = [lambda st=8 + pc: emit_vproj(st)]
                hooks[(2, 0, pc)] = [
                    lambda t=pc // 2, qk=pc % 2: emit_qkproj(3, t, qk)
                ]
                hooks[(2, 1, pc)] = [lambda st=12 + pc: emit_vproj(st)]

            # --- attention per q-chunk ---
            for qi in range(NQ):
                q_sl = slice(qi * QW, (qi + 1) * QW)
                nk = 4 * (qi + 1)
                for h in range(HPC):
                    t, p0 = h // 2, 64 * (h % 2)
                    cps = pctx.tile([DH + 1, QW], F32, tag="cps")
                    for pc in range(nk // 2):
                        d0 = 2 * pc - 4 * qi
                        c0p = max(0, 256 * pc - 512 * qi)
                        sps = ppair.tile([128, 2, QW], F32, tag="sps")
                        sc_off = None if qi == 0 else (96 if pc == 0 else 72)
                        with tc.high_priority(offset=sc_off):
                            for half in range(2):
                                kc = 2 * pc + half
                                nc.tensor.matmul(
                                    sps[:, half, c0p:QW],
                                    k8T[p0 : p0 + 64, t, :, kc * 128 : (kc + 1) * 128],
                                    q8T[p0 : p0 + 64, t, :, qi * QW + c0p : (qi + 1) * QW],
                                    start=True,
                                    stop=True,
                                    perf_mode=DR,
                                )
                        at = apool.tile([128, 2, QW], BF, tag="at")
                        with tc.high_priority(offset=sc_off):
                            nc.scalar.activation(
                                at[:, :, c0p:QW], sps[:, :, c0p:QW], EXP
                            )
                        if d0 >= 0:
                            m1 = min(128 * d0 + 256, QW)
                            nc.vector.tensor_mul(
                                at[:, :, c0p:m1],
                                at[:, :, c0p:m1],
                                msk[:, d0 : d0 + 2, c0p:m1],
                            )
                        for fn in hooks.get((qi, h, pc), ()):
                            fn()
                        if qi > 0 and h >= 1 and pc in (1, 2):
                            emit_wo_half((qi - 1) * 4 + h - 1, pc - 1)
                        for half in range(2):
                            kc = 2 * pc + half
                            c0 = max(0, 128 * kc - 512 * qi)
                            nc.tensor.matmul(
                                cps[:, c0:QW],
                                vv[:, kc, h, :],
                                at[:, half, c0:QW],
                                start=(kc == 0),
                                stop=(kc == nk - 1),
                            )
                    # softmax normalization: ctxT = (64 ctx) * (1/(64 den)),
                    # 1/den broadcast across the 64 dh partitions on GPSIMD
                    # (tensor_tensor cannot take two PSUM operands).
                    nrm_off = 64 if qi == NQ - 1 else 0
                    with tc.high_priority(offset=nrm_off):
                        rc = rpool.tile([1, QW], F32R)
                        with nc.allow_low_precision(reason="f32r bits ~ f32"):
                            nc.vector.reciprocal(rc[:], cps[DH : DH + 1, :])
                        rc64 = bpool.tile([64, QW], F32R)
                        nc.gpsimd.partition_broadcast(rc64[:], rc[:], channels=64)
                        nc.vector.tensor_mul(
                            ctxT[p0 : p0 + 64, t, q_sl], cps[0:DH, :], rc64[:]
                        )
                if qi > 0:
                    emit_wo(qi * 4 - 1)
            with tc.high_priority(offset=64):
                for j in range(4):
                    emit_wo(12 + j, tail=True)


def build_program():
    if "nc" in _CACHE:
        return _CACHE["nc"]
    nc = bacc.Bacc(
        "TRN2", target_bir_lowering=False, debug=False, num_devices=N_CORES
    )
    _emit(nc)
    nc.compile()
    _CACHE["nc"] = nc
    return nc


def _pack_dr(a):
    """[D, N] -> [128, D//256, 2, N] with D-index = cc*256 + j*128 + p."""
    n = a.shape[1]
    return np.ascontiguousarray(
        a.reshape(D // 256, 2, 128, n).transpose(2, 0, 1, 3)
    )


def _pack_x(a):
    """[D, S] -> [128, NQ, NCC, 2, QW]: DR pack + 512-col phase outermost."""
    return np.ascontiguousarray(
        _pack_dr(a).reshape(128, NCC, 2, NQ, QW).transpose(0, 3, 1, 2, 4)
    )


def _prep_in_maps(query, key, value, Wq, Wk, Wv, Wo):
    bf = ml_dtypes.bfloat16
    f8 = ml_dtypes.float8_e4m3

    p, i, j = np.ogrid[0:128, 0:4, 0:QW]
    msk = (j >= 128 * i + p).astype(bf)

    xT = {}
    for b in range(B):
        x8q = np.ascontiguousarray(query[b].T).astype(f8)
        x8k = np.ascontiguousarray(key[b].T).astype(f8)
        xT[("qk", b)] = np.ascontiguousarray(
            np.stack([_pack_x(x8q), _pack_x(x8k)], axis=2)
        )
        tv = np.ascontiguousarray(value[b].T)
        x8v = tv.astype(f8)
        dx16 = ((tv - x8v.astype(np.float32)) * 16.0).astype(f8)
        xT[("vv", b)] = np.ascontiguousarray(
            np.stack([_pack_x(x8v), _pack_x(dx16)], axis=2)
        )

    in_maps = []
    for c in range(N_CORES):
        b, g = c // HPC, c % HPC
        rows = slice(g * DG, (g + 1) * DG)
        wqT = np.ascontiguousarray(Wq[rows].T) * 16.0  # [D, DG]
        wkT = np.ascontiguousarray(Wk[rows].T) * 32.0
        wvT = np.ascontiguousarray(Wv[rows].T) * 64.0
        wv8 = wvT.astype(f8)
        dwv8 = (wvT - wv8.astype(np.float32)).astype(f8)
        wv4 = (wvT / 16.0).astype(f8)
        in_maps.append(
            {
                "xqk8": xT[("qk", b)],
                "xvv8": xT[("vv", b)],
                "wqk8": np.ascontiguousarray(
                    np.stack(
                        [_pack_dr(wqT.astype(f8)), _pack_dr(wkT.astype(f8))],
                        axis=1,
                    )  # [128, qk, cc, j, DG]
                    .reshape(128, 2, NCC, 2, 2, 128)
                    .transpose(0, 4, 1, 2, 3, 5)  # [128, t, qk, cc, j, 128]
                ),
                "wv3": np.ascontiguousarray(
                    np.stack(
                        [_pack_dr(wv8), _pack_dr(dwv8), _pack_dr(wv4)], axis=1
                    )
                ),
                "woT": np.ascontiguousarray(Wo[:, rows].T)
                .astype(bf)
                .reshape(2, 128, D)
                .transpose(1, 0, 2)
                .copy(),
                "msk": msk,
            }
        )
    return in_maps


def kernel(query, key, value, Wq, Wk, Wv, Wo):
    query = np.asarray(query, dtype=np.float32)
    key = np.asarray(key, dtype=np.float32)
    value = np.asarray(value, dtype=np.float32)
    Wq = np.asarray(Wq, dtype=np.float32)
    Wk = np.asarray(Wk, dtype=np.float32)
    Wv = np.asarray(Wv, dtype=np.float32)
    Wo = np.asarray(Wo, dtype=np.float32)

    nc = build_program()
    in_maps = _prep_in_maps(query, key, value, Wq, Wk, Wv, Wo)
    res = run_bass_kernel_spmd(
        nc, in_maps, core_ids=list(range(N_CORES)), trace=False
    )
    out = np.zeros((B, S, D), dtype=np.float32)
    for b in range(B):
        for g in range(HPC):
            out[b] += res.results[b * HPC + g]["outp"].astype(np.float32)
    return out


# revision 9
# speedup vs baseline: 1.3803x; 1.0171x over previous
"""Multi-head causal attention (B=2, S=2048, D=1024, H=16, DH=64) on 8 NeuronCores.

Sharding: data-parallel over batch (2) x tensor-parallel over heads (4 groups
of 4 heads). Core c handles batch c//4, heads 4*(c%4)..4*(c%4)+3. Host sums
the 4 Wo row-shard partials per batch.

Precision/engine plan (validated numerically against the reference):
- Q/K projections: direct fp8(e4m3) DoubleRow matmuls (K=256/instr). Host
  uploads x8 = fp8(x) and Wq*16 / Wk*32 in fp8; PSUM holds 16*q / 32*k and the
  DVE lift scales by 1/64 into fp8 score operands q/4 and k/2, so the q*k
  matmul directly yields q*k/8 (the softmax scale).
- Scores: fp8 DoubleRow, contraction dh=64 on a 64-partition slice with the
  second DR k-tile zeroed.
- V projection: 3-term compensated fp8 DoubleRow (x8@W64 + x8@dW + 16dx@4W)
  keeping v at ~bf16 accuracy; PSUM holds 64*v which cancels against the
  1/(64*den) reciprocal in the softmax normalization.
- attention*V, Wo: bf16 (fp8 here fails the error budget).
- exp on ACT; causal mask multiply on GPSIMD; 1/den broadcast by rank-1 PE
  matmul; PSUM->SBUF lifts and output copies on DVE.

Dataflow choices for pipeline fill: q/k (and v/dv) inputs ship as single
combined dram tensors DMA'd in column phases (cols 0:512 first) so the first
score pairs are ready ~5us in; projections emit si-major (q then k per si) so
attention on q-chunk 0 unblocks after 4 of 16 projection tiles; v-projection
chunks are spread inside the first head's pair loop of each q-chunk so the
ACT exp stream never starves behind a projection burst.
"""

import numpy as np
import ml_dtypes

import concourse.bass as bass  # noqa: F401
import concourse.mybir as mybir
import concourse.tile as tile
from concourse import bacc
from concourse.bass_utils import run_bass_kernel_spmd

B, S, D, H, DH = 2, 2048, 1024, 16, 64
N_CORES = 8
HPC = 4            # heads per core
DG = HPC * DH      # 256 head dims per core
QW = 512           # q-chunk width
NQ = S // QW       # 4 q-chunks
NCC = D // 256     # 4 DR contraction chunks for projections

BF = mybir.dt.bfloat16
F8 = mybir.dt.float8e4
F32 = mybir.dt.float32
F32R = mybir.dt.float32r
DR = mybir.MatmulPerfMode.DoubleRow

_CACHE = {}


def _emit(nc):
    # x layouts put the 512-wide column phase OUTERMOST so each phase DMA is
    # one contiguous per-partition subregion (cheap exact dependency ranges).
    xqkd = nc.dram_tensor("xqk8", [128, NQ, 2, NCC, 2, QW], F8, kind="ExternalInput")
    xvvd = nc.dram_tensor("xvv8", [128, NQ, 2, NCC, 2, QW], F8, kind="ExternalInput")
    wqkd = nc.dram_tensor("wqk8", [128, 2, 2, NCC, 2, 128], F8, kind="ExternalInput")
    wv3d = nc.dram_tensor("wv3", [128, 3, NCC, 2, DG], F8, kind="ExternalInput")
    wod = nc.dram_tensor("woT", [128, 2, D], BF, kind="ExternalInput")
    mskd = nc.dram_tensor("msk", [128, 4, QW], BF, kind="ExternalInput")
    outp = nc.dram_tensor("outp", [S, D], BF, kind="ExternalOutput")

    EXP = mybir.ActivationFunctionType.Exp

    with tile.TileContext(nc) as tc:
        with (
            tc.tile_pool(name="wpool", bufs=1) as wpool,
            tc.tile_pool(name="spool", bufs=1) as spool,
            tc.tile_pool(name="apool", bufs=20) as apool,
            tc.tile_pool(name="rpool", bufs=6) as rpool,
            tc.tile_pool(name="bpool", bufs=6) as bpool,
            tc.tile_pool(name="opool", bufs=6) as opool,
            tc.tile_pool(name="ppair", bufs=2, space="PSUM") as ppair,
            tc.tile_pool(name="pmain", bufs=2, space="PSUM") as pmain,
            tc.tile_pool(name="pctx", bufs=2, space="PSUM") as pctx,
        ):
            # --- persistent tiles ---
            wqk8 = wpool.tile([128, 2, 2, NCC, 2, 128], F8)
            wv3 = wpool.tile([128, 3, NCC, 2, DG], F8)
            wo = wpool.tile([128, 2, D], BF)
            msk = wpool.tile([128, 4, QW], BF)
            xqk8 = spool.tile([128, NQ, 2, NCC, 2, QW], F8)
            xvv8 = spool.tile([128, NQ, 2, NCC, 2, QW], F8)
            # q/k score operands: [128 part(dh of t-half), t, ktile j, S];
            # j=1 is zero (DR pads contraction 64 -> 128).
            q8T = spool.tile([128, 2, 2, S], F8)
            k8T = spool.tile([128, 2, 2, S], F8)
            vv = spool.tile([128, S // 128, HPC, DH + 1], BF)
            ctxT = spool.tile([128, 2, S], BF)

            # column-phased input DMAs, ordered by first use: q-chunk 0's
            # scores, then its v-projection, then later phases.
            nc.sync.dma_start(wqk8[:, 0], wqkd.ap()[:, 0])
            nc.sync.dma_start(xqk8[:, 0, 0], xqkd.ap()[:, 0, 0])
            nc.sync.dma_start(xqk8[:, 0, 1], xqkd.ap()[:, 0, 1])
            nc.sync.dma_start(wqk8[:, 1], wqkd.ap()[:, 1])
            nc.sync.dma_start(xqk8[:, 1], xqkd.ap()[:, 1])
            nc.sync.dma_start(msk[:], mskd.ap())
            nc.sync.dma_start(wv3[:], wv3d.ap())
            nc.sync.dma_start(xvv8[:, 0], xvvd.ap()[:, 0])
            nc.sync.dma_start(xvv8[:, 1], xvvd.ap()[:, 1])
            nc.sync.dma_start(wo[:], wod.ap())
            nc.sync.dma_start(xqk8[:, 2], xqkd.ap()[:, 2])
            nc.sync.dma_start(xvv8[:, 2], xvvd.ap()[:, 2])
            nc.sync.dma_start(xqk8[:, 3], xqkd.ap()[:, 3])
            nc.sync.dma_start(xvv8[:, 3], xvvd.ap()[:, 3])

            # zero the j=1 DR slots of q8T/k8T (u32-bitcast for packed memset)
            nc.gpsimd.memset(q8T[:, :, 1, :].bitcast(mybir.dt.uint32), 0)
            nc.gpsimd.memset(k8T[:, :, 1, :].bitcast(mybir.dt.uint32), 0)
            nc.gpsimd.memset(vv[:, :, :, DH : DH + 1], 64.0)

            # --- q/k projections (direct fp8 DR) + fp8 lift ---
            # one (t, qk) tile; emitted just-in-time: si 0,1 upfront, si 2/3
            # hooked into attention on q-chunks 1/2 (after their DMA phase
            # lands) so the in-order PE stream never parks on a late phase.
            def emit_qkproj(si, t, qk):
                dst = q8T if qk == 0 else k8T
                ps = pmain.tile([128, QW], F32, tag="ps")
                off = 140 if si > 0 else 0
                with tc.high_priority(offset=off):
                    for cc in range(NCC):
                        nc.tensor.matmul(
                            ps[:],
                            wqk8[:, t, qk, cc, :, :],
                            xqk8[:, si, qk, cc, :, :],
                            start=(cc == 0),
                            stop=(cc == NCC - 1),
                            perf_mode=DR,
                        )
                with tc.high_priority(offset=off + 48):
                    nc.vector.tensor_scalar_mul(
                        dst[:, t, 0, si * QW : (si + 1) * QW],
                        ps[:],
                        1.0 / 64.0,
                    )

            for t in range(2):
                for qk in (0, 1):
                    emit_qkproj(0, t, qk)

            # --- v projection: 3-term compensated fp8 DR; vv holds 64*v ---
            def emit_vproj(st, pool=None, tag=None):
                sh, lc = st // 4, (st % 4) * 128
                if pool is None:
                    ps = pmain.tile([128, DG], F32, tag="ps")
                else:
                    ps = pool.tile([128, DG], F32, tag=tag)
                for term in range(3):
                    sel = 1 if term == 2 else 0
                    for cc in range(NCC):
                        nc.tensor.matmul(
                            ps[:],
                            xvv8[:, sh, sel, cc, :, lc : lc + 128],
                            wv3[:, term, cc, :, :],
                            start=(term == 0 and cc == 0),
                            stop=(term == 2 and cc == NCC - 1),
                            perf_mode=DR,
                        )
                nc.vector.tensor_copy(
                    vv[:, st, :, 0:DH],
                    ps[:].rearrange("p (h e) -> p h e", e=DH),
                )

            # --- output projection (bf16) ---
            ob_tiles = {}

            def emit_wo_half(qt, nh, tail=False):
                if nh == 0:
                    ob = opool.tile([128, D], BF, tag="ob")
                    ob_tiles[qt] = ob
                ob = ob_tiles[qt]
                if tail and (2 * qt + nh) % 2:
                    # last q-chunk: attention PSUM pools are idle; borrow one
                    # so four Wo accumulations can be in flight
                    ops = pctx.tile([128, 512], F32, tag="cps")
                else:
                    ops = pmain.tile([128, 512], F32, tag="ps")
                for t in range(2):
                    nc.tensor.matmul(
                        ops[:],
                        ctxT[:, t, qt * 128 : (qt + 1) * 128],
                        wo[:, t, nh * 512 : (nh + 1) * 512],
                        start=(t == 0),
                        stop=(t == 1),
                    )
                if tail and qt % 2:
                    # split the PSUM lift between DVE and the now-idle ACT
                    nc.scalar.copy(ob[:, nh * 512 : (nh + 1) * 512], ops[:])
                else:
                    nc.vector.tensor_copy(
                        ob[:, nh * 512 : (nh + 1) * 512], ops[:]
                    )
                if nh == 1:
                    nc.sync.dma_start(
                        outp.ap()[qt * 128 : (qt + 1) * 128, :], ob[:]
                    )
                    del ob_tiles[qt]

            def emit_wo(qt, tail=False):
                emit_wo_half(qt, 0, tail)
                emit_wo_half(qt, 1, tail)

            # just-in-time projection work, spread between score pairs so the
            # in-order PE stream never parks the exp feed behind a projection
            # burst: each (qi, h, pc) slot runs at most one chunk, placed a
            # q-chunk ahead of its consumer where possible.
            # hook order matters: pmain pool slots recycle in EMISSION
            # order, so next-q-chunk score projections take the earliest
            # slots (their consumers unblock the exp stream) and the
            # v-projections come after (the at-pool buffers cover their
            # consumers' latency).
            hooks = {}
            hooks.update(
                {

                    (0, 1, 0): [lambda: emit_qkproj(1, 0, 0)],
                    (0, 1, 1): [lambda: emit_qkproj(1, 0, 1)],
                    (0, 2, 0): [lambda: emit_qkproj(1, 1, 0)],
                    (0, 2, 1): [lambda: emit_qkproj(1, 1, 1)],
                }
            )
            for pc in range(4):
                hooks[(1, 0, pc)] = [lambda st=4 + pc: emit_vproj(st)]
                hooks[(1, 1, pc)] = [
                    lambda t=pc // 2, qk=pc % 2: emit_qkproj(2, t, qk)
                ]
                hooks[(1, 2, pc)] = [lambda st=8 + pc: emit_vproj(st)]
                hooks[(2, 0, pc)] = [
                    lambda t=pc // 2, qk=pc % 2: emit_qkproj(3, t, qk)
                ]
                hooks[(2, 1, pc)] = [lambda st=12 + pc: emit_vproj(st)]

            # --- attention per q-chunk ---
            for qi in range(NQ):
                q_sl = slice(qi * QW, (qi + 1) * QW)
                nk = 4 * (qi + 1)
                if qi == 0:
                    # q-chunk 0's v-projections run on the (still idle) pctx
                    # ring so the pmain ring feeds q-chunk 1's score
                    # projections as soon as their DMA phase lands
                    for st in range(4):
                        emit_vproj(st, pool=pctx, tag="cps")
                for h in range(HPC):
                    t, p0 = h // 2, 64 * (h % 2)
                    cps = pctx.tile([DH + 1, QW], F32, tag="cps")
                    for pc in range(nk // 2):
                        d0 = 2 * pc - 4 * qi
                        c0p = max(0, 256 * pc - 512 * qi)
                        sps = ppair.tile([128, 2, QW], F32, tag="sps")
                        sc_off = None if qi == 0 else (96 if pc == 0 else 72)
                        with tc.high_priority(offset=sc_off):
                            for half in range(2):
                                kc = 2 * pc + half
                                nc.tensor.matmul(
                                    sps[:, half, c0p:QW],
                                    k8T[p0 : p0 + 64, t, :, kc * 128 : (kc + 1) * 128],
                                    q8T[p0 : p0 + 64, t, :, qi * QW + c0p : (qi + 1) * QW],
                                    start=True,
                                    stop=True,
                                    perf_mode=DR,
                                )
                        at = apool.tile([128, 2, QW], BF, tag="at")
                        with tc.high_priority(offset=sc_off):
                            nc.scalar.activation(
                                at[:, :, c0p:QW], sps[:, :, c0p:QW], EXP
                            )
                        if d0 >= 0:
                            m1 = min(128 * d0 + 256, QW)
                            nc.vector.tensor_mul(
                                at[:, :, c0p:m1],
                                at[:, :, c0p:m1],
                                msk[:, d0 : d0 + 2, c0p:m1],
                            )
                        for fn in hooks.get((qi, h, pc), ()):
                            fn()
                        if qi > 0 and h >= 1 and pc in (1, 2):
                            emit_wo_half((qi - 1) * 4 + h - 1, pc - 1)
                        for half in range(2):
                            kc = 2 * pc + half
                            c0 = max(0, 128 * kc - 512 * qi)
                            nc.tensor.matmul(
                                cps[:, c0:QW],
                                vv[:, kc, h, :],
                                at[:, half, c0:QW],
                                start=(kc == 0),
                                stop=(kc == nk - 1),
                            )
                    # softmax normalization: ctxT = (64 ctx) * (1/(64 den)),
                    # 1/den broadcast across the 64 dh partitions on GPSIMD
                    # (tensor_tensor cannot take two PSUM operands).
                    nrm_off = 64 if qi == NQ - 1 else 0
                    with tc.high_priority(offset=nrm_off):
                        rc = rpool.tile([1, QW], F32R)
                        with nc.allow_low_precision(reason="f32r bits ~ f32"):
                            nc.vector.reciprocal(rc[:], cps[DH : DH + 1, :])
                        rc64 = bpool.tile([64, QW], F32R)
                        nc.gpsimd.partition_broadcast(rc64[:], rc[:], channels=64)
                        nc.vector.tensor_mul(
                            ctxT[p0 : p0 + 64, t, q_sl], cps[0:DH, :], rc64[:]
                        )
                if qi > 0:
                    emit_wo(qi * 4 - 1)
            with tc.high_priority(offset=64):
                for j in range(4):
                    emit_wo(12 + j, tail=True)


def build_program():
    if "nc" in _CACHE:
        return _CACHE["nc"]
    nc = bacc.Bacc(
        "TRN2", target_bir_lowering=False, debug=False, num_devices=N_CORES
    )
    _emit(nc)
    nc.compile()
    _CACHE["nc"] = nc
    return nc


def _pack_dr(a):
    """[D, N] -> [128, D//256, 2, N] with D-index = cc*256 + j*128 + p."""
    n = a.shape[1]
    return np.ascontiguousarray(
        a.reshape(D // 256, 2, 128, n).transpose(2, 0, 1, 3)
    )


def _pack_x(a):
    """[D, S] -> [128, NQ, NCC, 2, QW]: DR pack + 512-col phase outermost."""
    return np.ascontiguousarray(
        _pack_dr(a).reshape(128, NCC, 2, NQ, QW).transpose(0, 3, 1, 2, 4)
    )


def _prep_in_maps(query, key, value, Wq, Wk, Wv, Wo):
    bf = ml_dtypes.bfloat16
    f8 = ml_dtypes.float8_e4m3

    p, i, j = np.ogrid[0:128, 0:4, 0:QW]
    msk = (j >= 128 * i + p).astype(bf)

    xT = {}
    for b in range(B):
        x8q = np.ascontiguousarray(query[b].T).astype(f8)
        x8k = np.ascontiguousarray(key[b].T).astype(f8)
        xT[("qk", b)] = np.ascontiguousarray(
            np.stack([_pack_x(x8q), _pack_x(x8k)], axis=2)
        )
        tv = np.ascontiguousarray(value[b].T)
        x8v = tv.astype(f8)
        dx16 = ((tv - x8v.astype(np.float32)) * 16.0).astype(f8)
        xT[("vv", b)] = np.ascontiguousarray(
            np.stack([_pack_x(x8v), _pack_x(dx16)], axis=2)
        )

    in_maps = []
    for c in range(N_CORES):
        b, g = c // HPC, c % HPC
        rows = slice(g * DG, (g + 1) * DG)
        wqT = np.ascontiguousarray(Wq[rows].T) * 16.0  # [D, DG]
        wkT = np.ascontiguousarray(Wk[rows].T) * 32.0
        wvT = np.ascontiguousarray(Wv[rows].T) * 64.0
        wv8 = wvT.astype(f8)
        dwv8 = (wvT - wv8.astype(np.float32)).astype(f8)
        wv4 = (wvT / 16.0).astype(f8)
        in_maps.append(
            {
                "xqk8": xT[("qk", b)],
                "xvv8": xT[("vv", b)],
                "wqk8": np.ascontiguousarray(
                    np.stack(
                        [_pack_dr(wqT.astype(f8)), _pack_dr(wkT.astype(f8))],
                        axis=1,
                    )  # [128, qk, cc, j, DG]
                    .reshape(128, 2, NCC, 2, 2, 128)
                    .transpose(0, 4, 1, 2, 3, 5)  # [128, t, qk, cc, j, 128]
                ),
                "wv3": np.ascontiguousarray(
                    np.stack(
                        [_pack_dr(wv8), _pack_dr(dwv8), _pack_dr(wv4)], axis=1
                    )
                ),
                "woT": np.ascontiguousarray(Wo[:, rows].T)
                .astype(bf)
                .reshape(2, 128, D)
                .transpose(1, 0, 2)
                .copy(),
                "msk": msk,
            }
        )
    return in_maps


def kernel(query, key, value, Wq, Wk, Wv, Wo):
    query = np.asarray(query, dtype=np.float32)
    key = np.asarray(key, dtype=np.float32)
    value = np.asarray(value, dtype=np.float32)
    Wq = np.asarray(Wq, dtype=np.float32)
    Wk = np.asarray(Wk, dtype=np.float32)
    Wv = np.asarray(Wv, dtype=np.float32)
    Wo = np.asarray(Wo, dtype=np.float32)

    nc = build_program()
    in_maps = _prep_in_maps(query, key, value, Wq, Wk, Wv, Wo)
    res = run_bass_kernel_spmd(
        nc, in_maps, core_ids=list(range(N_CORES)), trace=False
    )
    out = np.zeros((B, S, D), dtype=np.float32)
    for b in range(B):
        for g in range(HPC):
            out[b] += res.results[b * HPC + g]["outp"].astype(np.float32)
    return out


# revision 10
# speedup vs baseline: 1.4098x; 1.0214x over previous
"""Multi-head causal attention (B=2, S=2048, D=1024, H=16, DH=64) on 8 NeuronCores.

Sharding: data-parallel over batch (2) x tensor-parallel over heads (4 groups
of 4 heads). Core c handles batch c//4, heads 4*(c%4)..4*(c%4)+3. Host sums
the 4 Wo row-shard partials per batch.

Precision/engine plan (validated numerically against the reference):
- Q/K projections: direct fp8(e4m3) DoubleRow matmuls (K=256/instr). Host
  uploads x8 = fp8(x) and Wq*16 / Wk*32 in fp8; PSUM holds 16*q / 32*k and the
  DVE lift scales by 1/64 into fp8 score operands q/4 and k/2, so the q*k
  matmul directly yields q*k/8 (the softmax scale).
- Scores: fp8 DoubleRow, contraction dh=64 on a 64-partition slice with the
  second DR k-tile zeroed.
- V projection: 3-term compensated fp8 DoubleRow (x8@W64 + x8@dW + 16dx@4W)
  keeping v at ~bf16 accuracy; PSUM holds 64*v which cancels against the
  1/(64*den) reciprocal in the softmax normalization.
- attention*V, Wo: bf16 (fp8 here fails the error budget).
- exp on ACT; causal mask multiply on GPSIMD; 1/den broadcast by rank-1 PE
  matmul; PSUM->SBUF lifts and output copies on DVE.

Dataflow choices for pipeline fill: q/k (and v/dv) inputs ship as single
combined dram tensors DMA'd in column phases (cols 0:512 first) so the first
score pairs are ready ~5us in; projections emit si-major (q then k per si) so
attention on q-chunk 0 unblocks after 4 of 16 projection tiles; v-projection
chunks are spread inside the first head's pair loop of each q-chunk so the
ACT exp stream never starves behind a projection burst.
"""

import numpy as np
import ml_dtypes

import concourse.bass as bass  # noqa: F401
import concourse.mybir as mybir
import concourse.tile as tile
from concourse import bacc
from concourse.bass_utils import run_bass_kernel_spmd

B, S, D, H, DH = 2, 2048, 1024, 16, 64
N_CORES = 8
HPC = 4            # heads per core
DG = HPC * DH      # 256 head dims per core
QW = 512           # q-chunk width
NQ = S // QW       # 4 q-chunks
NCC = D // 256     # 4 DR contraction chunks for projections

BF = mybir.dt.bfloat16
F8 = mybir.dt.float8e4
F32 = mybir.dt.float32
F32R = mybir.dt.float32r
DR = mybir.MatmulPerfMode.DoubleRow

_CACHE = {}


def _emit(nc):
    # x layouts put the 512-wide column phase OUTERMOST so each phase DMA is
    # one contiguous per-partition subregion (cheap exact dependency ranges).
    xqkd = nc.dram_tensor("xqk8", [128, NQ, 2, NCC, 2, QW], F8, kind="ExternalInput")
    xvvd = nc.dram_tensor("xvv8", [128, NQ, 2, NCC, 2, QW], F8, kind="ExternalInput")
    wqkd = nc.dram_tensor("wqk8", [128, 2, 2, NCC, 2, 128], F8, kind="ExternalInput")
    wv3d = nc.dram_tensor("wv3", [128, 3, NCC, 2, DG], F8, kind="ExternalInput")
    wod = nc.dram_tensor("woT", [128, 2, D], BF, kind="ExternalInput")
    mskd = nc.dram_tensor("msk", [128, 4, QW], BF, kind="ExternalInput")
    outp = nc.dram_tensor("outp", [S, D], BF, kind="ExternalOutput")

    EXP = mybir.ActivationFunctionType.Exp

    with tile.TileContext(nc) as tc:
        with (
            tc.tile_pool(name="wpool", bufs=1) as wpool,
            tc.tile_pool(name="spool", bufs=1) as spool,
            tc.tile_pool(name="apool", bufs=20) as apool,
            tc.tile_pool(name="rpool", bufs=6) as rpool,
            tc.tile_pool(name="bpool", bufs=6) as bpool,
            tc.tile_pool(name="opool", bufs=6) as opool,
            tc.tile_pool(name="ppair", bufs=2, space="PSUM") as ppair,
            tc.tile_pool(name="pmain", bufs=2, space="PSUM") as pmain,
            tc.tile_pool(name="pctx", bufs=2, space="PSUM") as pctx,
        ):
            # --- persistent tiles ---
            wqk8 = wpool.tile([128, 2, 2, NCC, 2, 128], F8)
            wv3 = wpool.tile([128, 3, NCC, 2, DG], F8)
            wo = wpool.tile([128, 2, D], BF)
            msk = wpool.tile([128, 4, QW], BF)
            xqk8 = spool.tile([128, NQ, 2, NCC, 2, QW], F8)
            xvv8 = spool.tile([128, NQ, 2, NCC, 2, QW], F8)
            # q/k score operands: [128 part(dh of t-half), t, ktile j, S];
            # j=1 is zero (DR pads contraction 64 -> 128).
            q8T = spool.tile([128, 2, 2, S], F8)
            k8T = spool.tile([128, 2, 2, S], F8)
            vv = spool.tile([128, S // 128, HPC, DH + 1], BF)
            ctxT = spool.tile([128, 2, S], BF)

            # column-phased input DMAs, ordered by first use: q-chunk 0's
            # scores, then its v-projection, then later phases.
            nc.sync.dma_start(wqk8[:, 0], wqkd.ap()[:, 0])
            nc.sync.dma_start(xqk8[:, 0, 0], xqkd.ap()[:, 0, 0])
            nc.sync.dma_start(xqk8[:, 0, 1], xqkd.ap()[:, 0, 1])
            nc.sync.dma_start(wqk8[:, 1], wqkd.ap()[:, 1])
            nc.sync.dma_start(xqk8[:, 1], xqkd.ap()[:, 1])
            nc.sync.dma_start(msk[:], mskd.ap())
            nc.sync.dma_start(wv3[:], wv3d.ap())
            nc.sync.dma_start(xvv8[:, 0], xvvd.ap()[:, 0])
            nc.sync.dma_start(xvv8[:, 1], xvvd.ap()[:, 1])
            nc.sync.dma_start(wo[:], wod.ap())
            nc.sync.dma_start(xqk8[:, 2], xqkd.ap()[:, 2])
            nc.sync.dma_start(xvv8[:, 2], xvvd.ap()[:, 2])
            nc.sync.dma_start(xqk8[:, 3], xqkd.ap()[:, 3])
            nc.sync.dma_start(xvv8[:, 3], xvvd.ap()[:, 3])

            # zero the j=1 DR slots of q8T/k8T (u32-bitcast for packed memset)
            nc.gpsimd.memset(q8T[:, :, 1, :].bitcast(mybir.dt.uint32), 0)
            nc.gpsimd.memset(k8T[:, :, 1, :].bitcast(mybir.dt.uint32), 0)
            nc.gpsimd.memset(vv[:, :, :, DH : DH + 1], 64.0)

            # --- q/k projections (direct fp8 DR) + fp8 lift ---
            # one (t, qk) tile; emitted just-in-time: si 0,1 upfront, si 2/3
            # hooked into attention on q-chunks 1/2 (after their DMA phase
            # lands) so the in-order PE stream never parks on a late phase.
            def emit_qkproj(si, t, qk):
                dst = q8T if qk == 0 else k8T
                ps = pmain.tile([128, QW], F32, tag="ps")
                off = 140 if si > 0 else 0
                with tc.high_priority(offset=off):
                    for cc in range(NCC):
                        nc.tensor.matmul(
                            ps[:],
                            wqk8[:, t, qk, cc, :, :],
                            xqk8[:, si, qk, cc, :, :],
                            start=(cc == 0),
                            stop=(cc == NCC - 1),
                            perf_mode=DR,
                        )
                with tc.high_priority(offset=off + 48):
                    nc.vector.tensor_scalar_mul(
                        dst[:, t, 0, si * QW : (si + 1) * QW],
                        ps[:],
                        1.0 / 64.0,
                    )

            for t in range(2):
                for qk in (0, 1):
                    emit_qkproj(0, t, qk)

            # --- v projection: 3-term compensated fp8 DR; vv holds 64*v ---
            def emit_vproj(st, pool=None, tag=None):
                sh, lc = st // 4, (st % 4) * 128
                if pool is None:
                    ps = pmain.tile([128, DG], F32, tag="ps")
                else:
                    ps = pool.tile([128, DG], F32, tag=tag)
                for term in range(3):
                    sel = 1 if term == 2 else 0
                    for cc in range(NCC):
                        nc.tensor.matmul(
                            ps[:],
                            xvv8[:, sh, sel, cc, :, lc : lc + 128],
                            wv3[:, term, cc, :, :],
                            start=(term == 0 and cc == 0),
                            stop=(term == 2 and cc == NCC - 1),
                            perf_mode=DR,
                        )
                nc.vector.tensor_copy(
                    vv[:, st, :, 0:DH],
                    ps[:].rearrange("p (h e) -> p h e", e=DH),
                )

            # --- output projection (bf16) ---
            ob_tiles = {}

            def emit_wo_half(qt, nh, tail=False):
                if nh == 0:
                    ob = opool.tile([128, D], BF, tag="ob")
                    ob_tiles[qt] = ob
                ob = ob_tiles[qt]
                if tail and (2 * qt + nh) % 2:
                    # last q-chunk: attention PSUM pools are idle; borrow one
                    # so four Wo accumulations can be in flight
                    ops = pctx.tile([128, 512], F32, tag="cps")
                else:
                    ops = pmain.tile([128, 512], F32, tag="ps")
                for t in range(2):
                    nc.tensor.matmul(
                        ops[:],
                        ctxT[:, t, qt * 128 : (qt + 1) * 128],
                        wo[:, t, nh * 512 : (nh + 1) * 512],
                        start=(t == 0),
                        stop=(t == 1),
                    )
                if tail and qt % 2:
                    # split the PSUM lift between DVE and the now-idle ACT
                    nc.scalar.copy(ob[:, nh * 512 : (nh + 1) * 512], ops[:])
                else:
                    nc.vector.tensor_copy(
                        ob[:, nh * 512 : (nh + 1) * 512], ops[:]
                    )
                if nh == 1:
                    nc.sync.dma_start(
                        outp.ap()[qt * 128 : (qt + 1) * 128, :], ob[:]
                    )
                    del ob_tiles[qt]

            def emit_wo(qt, tail=False):
                emit_wo_half(qt, 0, tail)
                emit_wo_half(qt, 1, tail)

            # just-in-time projection work, spread between score pairs so the
            # in-order PE stream never parks the exp feed behind a projection
            # burst: each (qi, h, pc) slot runs at most one chunk, placed a
            # q-chunk ahead of its consumer where possible.
            # hook order matters: pmain pool slots recycle in EMISSION
            # order, so next-q-chunk score projections take the earliest
            # slots (their consumers unblock the exp stream) and the
            # v-projections come after (the at-pool buffers cover their
            # consumers' latency).
            hooks = {}
            hooks.update(
                {

                    (0, 1, 0): [lambda: emit_qkproj(1, 0, 0)],
                    (0, 1, 1): [lambda: emit_qkproj(1, 0, 1)],
                    (0, 2, 0): [lambda: emit_qkproj(1, 1, 0)],
                    (0, 2, 1): [lambda: emit_qkproj(1, 1, 1)],
                }
            )
            for pc in range(4):
                hooks[(1, 0, pc)] = [lambda st=4 + pc: emit_vproj(st)]
                hooks[(1, 1, pc)] = [
                    lambda t=pc // 2, qk=pc % 2: emit_qkproj(2, t, qk)
                ]
                hooks[(2, 0, pc)] = [
                    lambda t=pc // 2, qk=pc % 2: emit_qkproj(3, t, qk)
                ]
            # spread next-q-chunk vproj prefetches across two heads each so
            # the chains never bunch between one head's score pairs
            for i in range(2):
                hooks[(1, 2, 2 * i)] = [lambda st=8 + i: emit_vproj(st)]
                hooks[(1, 3, 2 * i)] = [lambda st=10 + i: emit_vproj(st)]
                hooks[(2, 1, 3 * i)] = [lambda st=12 + i: emit_vproj(st)]
                hooks[(2, 2, 3 * i)] = [lambda st=14 + i: emit_vproj(st)]

            # --- attention per q-chunk ---
            for qi in range(NQ):
                q_sl = slice(qi * QW, (qi + 1) * QW)
                nk = 4 * (qi + 1)
                if qi == 0:
                    # q-chunk 0's v-projections run on the (still idle) pctx
                    # ring so the pmain ring feeds q-chunk 1's score
                    # projections as soon as their DMA phase lands
                    for st in range(4):
                        emit_vproj(st, pool=pctx, tag="cps")
                for h in range(HPC):
                    t, p0 = h // 2, 64 * (h % 2)
                    cps = pctx.tile([DH + 1, QW], F32, tag="cps")
                    for pc in range(nk // 2):
                        d0 = 2 * pc - 4 * qi
                        c0p = max(0, 256 * pc - 512 * qi)
                        sps = ppair.tile([128, 2, QW], F32, tag="sps")
                        sc_off = None if qi == 0 else (96 if pc == 0 else 72)
                        with tc.high_priority(offset=sc_off):
                            for half in range(2):
                                kc = 2 * pc + half
                                nc.tensor.matmul(
                                    sps[:, half, c0p:QW],
                                    k8T[p0 : p0 + 64, t, :, kc * 128 : (kc + 1) * 128],
                                    q8T[p0 : p0 + 64, t, :, qi * QW + c0p : (qi + 1) * QW],
                                    start=True,
                                    stop=True,
                                    perf_mode=DR,
                                )
                        at = apool.tile([128, 2, QW], BF, tag="at")
                        with tc.high_priority(offset=sc_off):
                            nc.scalar.activation(
                                at[:, :, c0p:QW], sps[:, :, c0p:QW], EXP
                            )
                        if d0 >= 0:
                            m1 = min(128 * d0 + 256, QW)
                            nc.vector.tensor_mul(
                                at[:, :, c0p:m1],
                                at[:, :, c0p:m1],
                                msk[:, d0 : d0 + 2, c0p:m1],
                            )
                        for fn in hooks.get((qi, h, pc), ()):
                            fn()
                        if qi > 0 and h >= 1 and pc in (1, 2):
                            emit_wo_half((qi - 1) * 4 + h - 1, pc - 1)
                        for half in range(2):
                            kc = 2 * pc + half
                            c0 = max(0, 128 * kc - 512 * qi)
                            nc.tensor.matmul(
                                cps[:, c0:QW],
                                vv[:, kc, h, :],
                                at[:, half, c0:QW],
                                start=(kc == 0),
                                stop=(kc == nk - 1),
                            )
                    # softmax normalization: ctxT = (64 ctx) * (1/(64 den)),
                    # 1/den broadcast across the 64 dh partitions on GPSIMD
                    # (tensor_tensor cannot take two PSUM operands).
                    nrm_off = 64 if qi == NQ - 1 else 0
                    with tc.high_priority(offset=nrm_off):
                        rc = rpool.tile([1, QW], F32R)
                        with nc.allow_low_precision(reason="f32r bits ~ f32"):
                            nc.vector.reciprocal(rc[:], cps[DH : DH + 1, :])
                        rc64 = bpool.tile([64, QW], F32R)
                        nc.gpsimd.partition_broadcast(rc64[:], rc[:], channels=64)
                        nc.vector.tensor_mul(
                            ctxT[p0 : p0 + 64, t, q_sl], cps[0:DH, :], rc64[:]
                        )
                if qi > 0:
                    emit_wo(qi * 4 - 1)
            with tc.high_priority(offset=64):
                for j in range(4):
                    emit_wo(12 + j, tail=True)


def build_program():
    if "nc" in _CACHE:
        return _CACHE["nc"]
    nc = bacc.Bacc(
        "TRN2", target_bir_lowering=False, debug=False, num_devices=N_CORES
    )
    _emit(nc)
    nc.compile()
    _CACHE["nc"] = nc
    return nc


def _pack_dr(a):
    """[D, N] -> [128, D//256, 2, N] with D-index = cc*256 + j*128 + p."""
    n = a.shape[1]
    return np.ascontiguousarray(
        a.reshape(D // 256, 2, 128, n).transpose(2, 0, 1, 3)
    )


def _pack_x(a):
    """[D, S] -> [128, NQ, NCC, 2, QW]: DR pack + 512-col phase outermost."""
    return np.ascontiguousarray(
        _pack_dr(a).reshape(128, NCC, 2, NQ, QW).transpose(0, 3, 1, 2, 4)
    )


def _prep_in_maps(query, key, value, Wq, Wk, Wv, Wo):
    bf = ml_dtypes.bfloat16
    f8 = ml_dtypes.float8_e4m3

    p, i, j = np.ogrid[0:128, 0:4, 0:QW]
    msk = (j >= 128 * i + p).astype(bf)

    xT = {}
    for b in range(B):
        x8q = np.ascontiguousarray(query[b].T).astype(f8)
        x8k = np.ascontiguousarray(key[b].T).astype(f8)
        xT[("qk", b)] = np.ascontiguousarray(
            np.stack([_pack_x(x8q), _pack_x(x8k)], axis=2)
        )
        tv = np.ascontiguousarray(value[b].T)
        x8v = tv.astype(f8)
        dx16 = ((tv - x8v.astype(np.float32)) * 16.0).astype(f8)
        xT[("vv", b)] = np.ascontiguousarray(
            np.stack([_pack_x(x8v), _pack_x(dx16)], axis=2)
        )

    in_maps = []
    for c in range(N_CORES):
        b, g = c // HPC, c % HPC
        rows = slice(g * DG, (g + 1) * DG)
        wqT = np.ascontiguousarray(Wq[rows].T) * 16.0  # [D, DG]
        wkT = np.ascontiguousarray(Wk[rows].T) * 32.0
        wvT = np.ascontiguousarray(Wv[rows].T) * 64.0
        wv8 = wvT.astype(f8)
        dwv8 = (wvT - wv8.astype(np.float32)).astype(f8)
        wv4 = (wvT / 16.0).astype(f8)
        in_maps.append(
            {
                "xqk8": xT[("qk", b)],
                "xvv8": xT[("vv", b)],
                "wqk8": np.ascontiguousarray(
                    np.stack(
                        [_pack_dr(wqT.astype(f8)), _pack_dr(wkT.astype(f8))],
                        axis=1,
                    )  # [128, qk, cc, j, DG]
                    .reshape(128, 2, NCC, 2, 2, 128)
                    .transpose(0, 4, 1, 2, 3, 5)  # [128, t, qk, cc, j, 128]
                ),
                "wv3": np.ascontiguousarray(
                    np.stack(
                        [_pack_dr(wv8), _pack_dr(dwv8), _pack_dr(wv4)], axis=1
                    )
                ),
                "woT": np.ascontiguousarray(Wo[:, rows].T)
                .astype(bf)
                .reshape(2, 128, D)
                .transpose(1, 0, 2)
                .copy(),
                "msk": msk,
            }
        )
    return in_maps


def kernel(query, key, value, Wq, Wk, Wv, Wo):
    query = np.asarray(query, dtype=np.float32)
    key = np.asarray(key, dtype=np.float32)
    value = np.asarray(value, dtype=np.float32)
    Wq = np.asarray(Wq, dtype=np.float32)
    Wk = np.asarray(Wk, dtype=np.float32)
    Wv = np.asarray(Wv, dtype=np.float32)
    Wo = np.asarray(Wo, dtype=np.float32)

    nc = build_program()
    in_maps = _prep_in_maps(query, key, value, Wq, Wk, Wv, Wo)
    res = run_bass_kernel_spmd(
        nc, in_maps, core_ids=list(range(N_CORES)), trace=False
    )
    out = np.zeros((B, S, D), dtype=np.float32)
    for b in range(B):
        for g in range(HPC):
            out[b] += res.results[b * HPC + g]["outp"].astype(np.float32)
    return out
